# revision 48
# baseline (speedup 1.0000x reference)
"""Trainium2 Bass kernel for a single-head transformer layer (dense_transformer).

Reference math (fp32, unscaled single-head attention):
    Q = src@Wq+bq; K = src@Wk+bk; V = src@Wv+bv
    attn = softmax(Q@K^T) @ V @ Wo + bo
    x  = LN(src + attn)*g1 + be1
    out = LN(x + relu(x@W1+b1)@W2 + b2)*g2 + be2

Sharding: 8 cores = 4 batches x 2 query halves, no collectives. Each core
computes its 1024 query rows against the full 2048-token context of its
batch. srcT is column-PERMUTED per core so the core's own query half
occupies columns 0..1023 (attention is permutation-invariant over context
order).

Weight-fold restructure (the big win vs a direct mapping): because the
attention is single-head and unscaled,
    scores = (srcq@Wq + bq) @ (src@Wk)^T = srcq @ (Wq@Wk^T) @ src^T + bqk@src^T
    attn@Wo = softmax(scores) @ src @ (Wv@Wo)
so the host precomputes Wp = Wq@Wk^T and Wvo = Wv@Wo once (1024^2 each)
and the device never projects K or V at all:
    qT = Wp^T-contract(srcq)  [2.1 GF]   (replaces Q proj + K proj: 6.4 GF)
    S  = qT vs srcT directly  [4.3 GF]
    aT = src^T-contract(P)    [4.3 GF]   (replaces V proj + P@V: 8.6 GF)
    x  = aT^T @ Wvo           [2.1 GF]   (absorbs the O proj)
Per-core Tensor work drops from ~30 GF-bf16-equivalent to ~21.5.

Attention path runs bf16 (scores from bf16 Wp/src; P stored bf16 --
softmax renormalization is computed from the same bf16 P, so the
quantization cancels in the weighted average). The feedforward runs in
fp8 e4m3 with perf_mode=DoubleRow: W1/W2 are quantized host-side with
x16/x32 scaling; exact inverse scales applied in the relu epilogue and
FF2 eviction. All DRAM tensors are host PRE-TILED so every DMA is a
contiguous 2-32KB run per partition.

Exact host-side folds (no accuracy cost): bk dropped (softmax-invariant),
bv -> residual as bv@Wo, bo -> srcq, bq -> bqk = bq@Wk^T (per-partition
bias on qT), LN1 affine -> W1' = diag(g1)@W1, b1' = b1 + be1@W1, and
be1+b2 -> one residual vector. The kernel keeps only normalized x-hat
(bf16) out of LN1; the residual base x-hat*g1 + (be1+b2) is rebuilt in
phase D's Vector slack.

Per-core phases (Tensor stream is never intentionally idle):
  A: qT[e,q] = Wp.T-contract @ srcT_{0,1} + bqk   (Scalar Identity evict)
  B: per 512-chunk c: S^T = srcT_c-contract @ qT; pc = exp(S^T) (bf16);
     sums += 1^T @ pc (PSUM across all chunks); aT += srcK_c^T @ pc
     (SBUF bf16 accumulation). Then sums -> partitions via K=1 matmuls,
     reciprocal -> rsum.
  C: x = (aT^T @ Wvo)*rsum + (src+bo'); LN1 stats ride the eviction
     (accum_out + Scalar Square); x-hat = Scalar Identity(scale=rstd,
     bias=-mu*rstd) -> bf16; PE-transpose (bf16, 1 cyc/row) -> xT fp8.
     Transposes lag one row so the Tensor never waits on the LN chain.
  D: xbase = x-hat*g1 + (be1+b2); per query half: FF1 (fp8 DR,
     relu*1/16+b1' -> h fp8), then per 128-row: FF2 (fp8 DR, 16-matmul
     PSUM accumulation), eviction x = xbase + ps/32 emits LN2 sums via
     accum_out + Scalar Square; LN2 normalize on Scalar, *g2+be2 on
     Vector, store. Row epilogues hide under the next row's matmuls.
"""

import numpy as np
from contextlib import ExitStack

import concourse.bacc as bacc
import concourse.tile as tile
from concourse import mybir
from concourse.masks import make_identity

P = 128
E = 1024          # embed
F = 4096          # dff
S = 2048          # context length per batch
NQ = 1024         # query rows per core
ET = E // P       # 8
FT = F // P       # 32
QS = NQ // 512    # 2 query slices of 512
KCH = 512         # k-chunk size
NCH = S // KCH    # 4 chunks
KT = KCH // P     # 4 k-tiles per chunk
FCH = 4           # f-tiles per FF chunk (512 f-columns)
NFC = FT // FCH   # 8 FF chunks
W1_SCALE = 16.0   # host premultiplier on W1 before e4m3 cast
W2_SCALE = 32.0   # host premultiplier on W2 before e4m3 cast
f32 = mybir.dt.float32
bf16 = mybir.dt.bfloat16
f8 = mybir.dt.float8e4
u8 = mybir.dt.uint8
DR = mybir.MatmulPerfMode.DoubleRow
EPS = 1e-5

SUB = mybir.AluOpType.subtract
MULT = mybir.AluOpType.mult
ADD = mybir.AluOpType.add
AF = mybir.ActivationFunctionType


def _f8(ap):
    """uint8 DRAM bytes -> fp8e4m3 view (numpy has no fp8 dtype)."""
    return ap.bitcast(f8)


def build_program():
    nc = bacc.Bacc("TRN2", target_bir_lowering=False, debug=False, num_devices=8)

    srcTt = nc.dram_tensor("srcTt", [P, NCH, ET, KCH], bf16, kind="ExternalInput").ap()
    srcKt = nc.dram_tensor("srcKt", [P, NCH * KT, E], bf16, kind="ExternalInput").ap()
    srcq = nc.dram_tensor("srcq", [P, ET, E], bf16, kind="ExternalInput").ap()
    Wpt = nc.dram_tensor("Wpt", [P, ET, ET, P], bf16, kind="ExternalInput").ap()
    Wvot = nc.dram_tensor("Wvot", [P, ET, E], bf16, kind="ExternalInput").ap()
    W1t = nc.dram_tensor("W1t", [P, NFC, ET, 512], u8, kind="ExternalInput").ap()
    W2t = nc.dram_tensor("W2t", [P, NFC, FCH, E], u8, kind="ExternalInput").ap()
    # bqk | b1' pre-tiled host-side to [P, ET+FT]: element i at [i%128, i//128]
    bqb1 = nc.dram_tensor("bqb1", [P, ET + FT], f32, kind="ExternalInput").ap()
    beb2 = nc.dram_tensor("beb2", [E], f32, kind="ExternalInput").ap()
    g1 = nc.dram_tensor("g1", [E], f32, kind="ExternalInput").ap()
    g2 = nc.dram_tensor("g2", [E], f32, kind="ExternalInput").ap()
    be2 = nc.dram_tensor("be2", [E], f32, kind="ExternalInput").ap()
    out = nc.dram_tensor("out", [NQ, E], f32, kind="ExternalOutput").ap()

    with tile.TileContext(nc) as tc, ExitStack() as ctx:
        consts = ctx.enter_context(tc.tile_pool(name="consts", bufs=1))
        lnp = ctx.enter_context(tc.tile_pool(name="lnp", bufs=6))
        bcp = ctx.enter_context(tc.tile_pool(name="bcp", bufs=1))
        # long-lived tensors allocated BEFORE the phase-A/B pools so their
        # DMAs never alias (and never wait on) the attention working set
        aT_pool = ctx.enter_context(tc.tile_pool(name="aT_pool", bufs=1))
        wvo_pool = ctx.enter_context(tc.tile_pool(name="wvo", bufs=1))
        # 4-deep W1 ring: chunks stream through twice (once per query
        # half) instead of sitting fully resident -- saves 16KB/partition
        w1_pool = ctx.enter_context(tc.tile_pool(name="w1p", bufs=4))
        sq_pool = ctx.enter_context(tc.tile_pool(name="sqp", bufs=1))

        aT = aT_pool.tile([P, ET, NQ], bf16)
        wvo_sb = wvo_pool.tile([P, ET, E], bf16)
        # residual src+bo' prefetched bf16 during attention: phase C must
        # not wait on DMAs that queue behind the W2 prefetch burst
        sq_sb = sq_pool.tile([P, ET, E], bf16)

        bias_sb = consts.tile([P, ET + FT], f32)
        nc.sync.dma_start(out=bias_sb, in_=bqb1)
        bqk_sb = bias_sb[:, 0:ET]
        b1_sb = bias_sb[:, ET:]
        ones_bf = consts.tile([P, 1], bf16)
        nc.vector.memset(ones_bf, 1.0)
        one_sp = consts.tile([1, 1], f32)
        nc.vector.memset(one_sp, 1.0)
        eps_sb = consts.tile([P, 1], f32)
        nc.vector.memset(eps_sb, EPS)
        inv_w2s = consts.tile([P, 1], f32)
        nc.vector.memset(inv_w2s, 1.0 / W2_SCALE)
        sums_sb = consts.tile([1, NQ], f32)
        rsum = consts.tile([P, ET], f32)
        ident_bf = consts.tile([P, P], bf16)
        make_identity(nc, ident_bf)

        def bcast(vec, n, pool, eng=None):
            t = pool.tile([P, n], f32, tag=f"bc_{vec.tensor.name}")
            (eng or nc.sync).dma_start(out=t, in_=vec.partition_broadcast(P))
            return t

        def ln_from_asum(asum, tag, want_nmr=False):
            """mu/rstd (and optionally nmr = -mu*rstd for the Scalar-side
            normalize) from accumulated [s_x(lo), s_x(hi), s_x2(lo),
            s_x2(hi)]."""
            mu = lnp.tile([P, 1], f32, tag=f"mu{tag}")
            nc.vector.tensor_add(out=mu, in0=asum[:, 0:1], in1=asum[:, 1:2])
            nc.vector.tensor_scalar_mul(out=mu, in0=mu, scalar1=1.0 / E)
            s23 = lnp.tile([P, 1], f32, tag=f"s23{tag}")
            nc.vector.tensor_add(out=s23, in0=asum[:, 2:3], in1=asum[:, 3:4])
            mu2 = lnp.tile([P, 1], f32, tag=f"mu2{tag}")
            nc.vector.tensor_mul(out=mu2, in0=mu, in1=mu)
            vr = lnp.tile([P, 1], f32, tag=f"vr{tag}")
            nc.vector.tensor_scalar(out=vr, in0=s23, scalar1=1.0 / E,
                                    scalar2=mu2, op0=MULT, op1=SUB)
            rstd = lnp.tile([P, 1], f32, tag=f"rstd{tag}")
            nc.scalar.activation(out=rstd, in_=vr, func=AF.Sqrt,
                                 bias=eps_sb, scale=1.0)
            nc.vector.reciprocal(out=rstd, in_=rstd)
            if not want_nmr:
                return mu, rstd, None
            nmr = lnp.tile([P, 1], f32, tag=f"nmr{tag}")
            nc.vector.tensor_scalar(out=nmr, in0=mu, scalar1=rstd,
                                    scalar2=-1.0, op0=MULT, op1=MULT)
            return mu, rstd, nmr

        with ExitStack() as phAB:
            qT_pool = phAB.enter_context(tc.tile_pool(name="qT_pool", bufs=1))
            st_pool = phAB.enter_context(tc.tile_pool(name="stp", bufs=2))
            sta_pool = phAB.enter_context(tc.tile_pool(name="stap", bufs=2))
            stb_pool = phAB.enter_context(tc.tile_pool(name="stbp", bufs=2))
            sk_pool = phAB.enter_context(tc.tile_pool(name="skp", bufs=1))
            wp_pool = phAB.enter_context(tc.tile_pool(name="wpp", bufs=1))
            pc_pool = phAB.enter_context(tc.tile_pool(name="pcp", bufs=2))
            ps_s = phAB.enter_context(tc.tile_pool(name="ps_s", bufs=3, space="PSUM"))
            ps_a = phAB.enter_context(tc.tile_pool(name="ps_a", bufs=3, space="PSUM"))
            ps_sum = phAB.enter_context(tc.tile_pool(name="ps_sum", bufs=1, space="PSUM"))

            qT = qT_pool.tile([P, ET, NQ], bf16)
            sk = sk_pool.tile([P, NCH * KT, E], bf16)

            sums = []
            for qs in range(QS):
                sums.append(ps_sum.tile([1, 512], f32, tag=f"sums{qs}",
                                        name=f"sums{qs}"))

            sts, st_a, st_b = {}, {}, {}

            def st_load(cc, eng=None):
                eng = eng or nc.sync
                st = st_pool.tile([P, ET, KCH], bf16, tag="st", name=f"st{cc}")
                eng.dma_start(out=st, in_=srcTt[:, cc])
                sts[cc] = st

            def st_ap(cc, d_t):
                if cc in st_a:
                    return st_a[cc] if d_t == 0 else st_b[cc][:, d_t - 1, :]
                return sts[cc][:, d_t, :]

            def sk_load(cc, eng):
                eng.dma_start(out=sk[:, cc * KT:(cc + 1) * KT, :],
                              in_=srcKt[:, cc * KT:(cc + 1) * KT, :])

            # -- startup-critical DMAs (Sync queue, enqueued at t=0): ONLY
            # what phases A and early-B need, BATCHED into few instructions
            # (each DMA_DIRECT2D costs ~650ns of Sync-engine issue time; 33
            # separate descriptors serialized the whole startup window),
            # split into tiles sized to the consumption order. Everything
            # else enqueues from the Scalar engine's DGE queue at staged
            # points below, so the startup window isn't fair-shared away to
            # prefetches that aren't needed until t+100us.
            # dual-queue startup: Sync and Scalar DGE queues enqueue in
            # parallel (separate DMA engine groups), halving both the
            # ~650ns/instruction issue serialization and per-queue load
            wp0 = wp_pool.tile([P, ET, P], bf16, tag="wp0")
            wp1 = wp_pool.tile([P, ET, P], bf16, tag="wp1")
            wp23 = wp_pool.tile([P, 2, ET, P], bf16, tag="wp23")
            wp47 = wp_pool.tile([P, 4, ET, P], bf16, tag="wp47")
            st0a = sta_pool.tile([P, KCH], bf16, tag="sta", name="st0a")
            st0b = stb_pool.tile([P, ET - 1, KCH], bf16, tag="stb", name="st0b")
            st1a = sta_pool.tile([P, KCH], bf16, tag="sta", name="st1a")
            st1b = stb_pool.tile([P, ET - 1, KCH], bf16, tag="stb", name="st1b")
            st_a[0], st_b[0], st_a[1], st_b[1] = st0a, st0b, st1a, st1b
            nc.sync.dma_start(out=wp0, in_=Wpt[:, 0])
            nc.scalar.dma_start(out=st0b, in_=srcTt[:, 0, 1:, :])
            nc.sync.dma_start(out=st0a, in_=srcTt[:, 0, 0, :])
            nc.scalar.dma_start(out=st1a, in_=srcTt[:, 1, 0, :])
            nc.sync.dma_start(out=wp1, in_=Wpt[:, 1])
            nc.scalar.dma_start(out=st1b, in_=srcTt[:, 1, 1:, :])
            nc.sync.dma_start(out=wp23, in_=Wpt[:, 2:4])
            nc.sync.dma_start(out=wp47, in_=Wpt[:, 4:])
            nc.scalar.dma_start(out=sk[:, 0:KT, :], in_=srcKt[:, 0:KT, :])
            wp_ts = ([wp0, wp1] + [wp23[:, i] for i in range(2)]
                     + [wp47[:, i] for i in range(4)])

            # ---------------- Phase A: qT = Wp-contract(srcq) ----------------
            # (e_t=0, qs=1) deferred one iteration so the first matmuls only
            # need st0 -- st1 is still streaming in at that point.
            def qproj(e_t, qs):
                ps = ps_s.tile([P, 512], f32, tag="ps")
                for d_t in range(ET):
                    nc.tensor.matmul(ps, wp_ts[e_t][:, d_t, :], st_ap(qs, d_t),
                                     start=(d_t == 0), stop=(d_t == ET - 1))
                nc.scalar.activation(out=qT[:, e_t, qs * 512:(qs + 1) * 512],
                                     in_=ps, func=AF.Identity,
                                     bias=bqk_sb[:, e_t:e_t + 1], scale=1.0)

            for e_t in range(ET):
                qproj(e_t, 0)
                if e_t == 1:
                    qproj(0, 1)
                if e_t >= 1:
                    qproj(e_t, 1)

            # deferred prefetch, stage 1 (enqueues after A's evictions)
            st_load(2, eng=nc.scalar)
            sk_load(1, nc.scalar)

            # ---------------- Phase B: scores -> exp -> sums/aT --------------
            for cc in range(NCH):
                pc = pc_pool.tile([P, KT, NQ], bf16, tag="pc", name=f"pc{cc}")
                for qs in range(QS):
                    for kt in range(KT):
                        ps = ps_s.tile([P, 512], f32, tag="ps")
                        for d_t in range(ET):
                            nc.tensor.matmul(
                                ps, st_ap(cc, d_t)[:, kt * P:(kt + 1) * P],
                                qT[:, d_t, qs * 512:(qs + 1) * 512],
                                start=(d_t == 0), stop=(d_t == ET - 1))
                        nc.scalar.activation(
                            out=pc[:, kt, qs * 512:(qs + 1) * 512], in_=ps,
                            func=AF.Exp)
                        nc.tensor.matmul(sums[qs], ones_bf,
                                         pc[:, kt, qs * 512:(qs + 1) * 512],
                                         start=(cc == 0 and kt == 0),
                                         stop=(cc == NCH - 1 and kt == KT - 1))
                # deferred prefetch stages 2-4: enqueue from the Scalar
                # queue right after this chunk's exp evictions
                if cc == 0:
                    st_load(3, eng=nc.scalar)
                    sk_load(2, nc.scalar)
                elif cc == 1:
                    sk_load(3, nc.scalar)
                    nc.scalar.dma_start(out=wvo_sb[:, 0:4], in_=Wvot[:, 0:4])
                    nc.scalar.dma_start(out=wvo_sb[:, 4:], in_=Wvot[:, 4:])
                elif cc == 2:
                    nc.scalar.dma_start(out=sq_sb[:, 0:4], in_=srcq[:, 0:4])
                    nc.scalar.dma_start(out=sq_sb[:, 4:], in_=srcq[:, 4:])
                    g1_bc = bcast(g1, E, bcp, nc.scalar)
                    beb2_bc = bcast(beb2, E, bcp, nc.scalar)
                    g2_bc = bcast(g2, E, bcp, nc.scalar)
                    be2_bc = bcast(be2, E, bcp, nc.scalar)
                    w1_pre = []
                    for fc in range(4):
                        w1c = w1_pool.tile([P, ET, 512], f8, tag="w1",
                                           name=f"w1c{fc}")
                        nc.scalar.dma_start(out=w1c, in_=_f8(W1t[:, fc]))
                        w1_pre.append(w1c)
                if cc == NCH - 1:
                    # softmax denominators: spread sums[1, q] across
                    # partitions via K=1 matmuls (1-partition DMAs fail NEFF
                    # load). Hoisted BEFORE the last chunk's aT block so the
                    # rsum chain completes under it and phase C starts hot.
                    for qs in range(QS):
                        nc.vector.tensor_copy(
                            out=sums_sb[:, qs * 512:(qs + 1) * 512],
                            in_=sums[qs])
                    for t in range(ET):
                        pst = ps_a.tile([P, 1], f32, tag="ps", name=f"spread{t}")
                        nc.tensor.matmul(pst, sums_sb[0:1, t * P:(t + 1) * P],
                                         one_sp, start=True, stop=True)
                        nc.vector.tensor_copy(out=rsum[:, t:t + 1], in_=pst)
                    nc.vector.reciprocal(out=rsum, in_=rsum)
                for qs in range(QS):
                    for d_t in range(ET):
                        ps = ps_a.tile([P, 512], f32, tag="ps")
                        for kt in range(KT):
                            nc.tensor.matmul(
                                ps, sk[:, cc * KT + kt, d_t * P:(d_t + 1) * P],
                                pc[:, kt, qs * 512:(qs + 1) * 512],
                                start=(kt == 0), stop=(kt == KT - 1))
                        dst = aT[:, d_t, qs * 512:(qs + 1) * 512]
                        if cc == 0:
                            nc.vector.tensor_copy(out=dst, in_=ps)
                        else:
                            nc.vector.tensor_add(out=dst, in0=dst, in1=ps)

        # late pools: land in the freed attention SBUF; their DMAs start
        # as soon as the aliased phase-A/B tiles' last reads complete
        late = ExitStack()
        try:
            x_pool = late.enter_context(tc.tile_pool(name="x_pool", bufs=1))
            xb_pool = late.enter_context(tc.tile_pool(name="xb_pool", bufs=1))
            xT_pool = late.enter_context(tc.tile_pool(name="xT_pool", bufs=1))
            w2_pool = late.enter_context(tc.tile_pool(name="w2p", bufs=NFC))
            x_sb = x_pool.tile([P, ET, E], f32)    # [q(8x128), e] residual base
            xb16 = xb_pool.tile([P, ET, E], bf16)  # normalized x-hat
            xT = xT_pool.tile([P, ET, NQ], f8)     # [e, q] fp8 for the FF

            w2cs = []
            for fc in range(NFC):
                w2c = w2_pool.tile([P, FCH, E], f8, tag="w2", name=f"w2c{fc}")
                nc.sync.dma_start(out=w2c, in_=_f8(W2t[:, fc]))
                w2cs.append(w2c)

            # ---------- Phase C: x = aT^T@Wvo + residual, LN1, transpose -----
            with ExitStack() as ph3:
                sqs_pool = ph3.enter_context(tc.tile_pool(name="sqs", bufs=2))
                ps_o = ph3.enter_context(tc.tile_pool(name="ps_o", bufs=3, space="PSUM"))
                ps_t = ph3.enter_context(tc.tile_pool(name="ps_t", bufs=4, space="PSUM"))

                def transposes(q_t):
                    """xb16 row q_t -> xT (bf16 PE transposes, fp8 evict)."""
                    for ep in range(ET // 2):
                        pst = ps_t.tile([P, 2, P], bf16, tag="pst")
                        for j in range(2):
                            nc.tensor.transpose(
                                pst[:, j, :],
                                xb16[:, q_t, (2 * ep + j) * P:(2 * ep + j + 1) * P],
                                ident_bf)
                        nc.vector.tensor_copy(
                            out=xT[:, 2 * ep:2 * ep + 2, q_t * P:(q_t + 1) * P],
                            in_=pst)

                for q_t in range(ET):
                    asum = lnp.tile([P, 4], f32, tag="asumC", name=f"asumC{q_t}")
                    for eo in range(2):
                        ps = ps_o.tile([P, 512], f32, tag="ps")
                        for d_t in range(ET):
                            nc.tensor.matmul(ps, aT[:, d_t, q_t * P:(q_t + 1) * P],
                                             wvo_sb[:, d_t, eo * 512:(eo + 1) * 512],
                                             start=(d_t == 0), stop=(d_t == ET - 1))
                        dst = x_sb[:, q_t, eo * 512:(eo + 1) * 512]
                        # x = O*rsum + (src+bo'), one fused Vector op; emits
                        # sum(x) per half for LN1 (bo' = bo + bv@Wo, on host)
                        nc.vector.scalar_tensor_tensor(
                            out=dst, in0=ps, scalar=rsum[:, q_t:q_t + 1],
                            in1=sq_sb[:, q_t, eo * 512:(eo + 1) * 512],
                            op0=MULT, op1=ADD, accum_out=asum[:, eo:eo + 1])
                        sqscr = sqs_pool.tile([P, 512], f32, tag="sqs")
                        nc.scalar.activation(out=sqscr, in_=dst, func=AF.Square,
                                             accum_out=asum[:, 2 + eo:3 + eo])
                    _, rstd, nmr = ln_from_asum(asum, "C", want_nmr=True)
                    # normalize-only LN1 (affine folded into W1/b1'), bf16
                    nc.scalar.activation(out=xb16[:, q_t, :], in_=x_sb[:, q_t, :],
                                         func=AF.Identity, bias=nmr, scale=rstd)
                    # transposes lag one row so Tensor never waits on LN1
                    if q_t > 0:
                        transposes(q_t - 1)
                transposes(ET - 1)

            # ---------------- Phase D: feedforward (fp8 DR) + LN2 -----------
            with ExitStack() as ph4:
                h_pool = ph4.enter_context(tc.tile_pool(name="hp", bufs=1))
                sqs2_pool = ph4.enter_context(tc.tile_pool(name="sqs2", bufs=2))
                ps_h = ph4.enter_context(tc.tile_pool(name="ps_h", bufs=2, space="PSUM"))
                ps_f = ph4.enter_context(tc.tile_pool(name="ps_f", bufs=2, space="PSUM"))

                # rebuild the residual base x-hat*g1 + (be1+b2) in FF1's
                # Vector slack (x_sb rows are free post-normalize)
                for q_t in range(ET):
                    row = x_sb[:, q_t, :]
                    nc.vector.tensor_mul(out=row, in0=xb16[:, q_t, :], in1=g1_bc)
                    nc.vector.tensor_add(out=row, in0=row, in1=beb2_bc)

                # W1 ring refills on the Scalar queue (lookahead 2): the
                # Sync queue head-of-line blocks behind LN2-gated output
                # stores, which would stall FF1(qs1)'s weight loads
                w1s = {g: w1_pre[g] for g in range(4)}

                def w1_refill(g):
                    if g > 15 or g in w1s:
                        return
                    t = w1_pool.tile([P, ET, 512], f8, tag="w1", name=f"w1g{g}")
                    nc.scalar.dma_start(out=t, in_=_f8(W1t[:, g % NFC]))
                    w1s[g] = t

                for qs in range(QS):
                    # FF1 for this query half: h = relu(ps/16 + b1') fp8.
                    # One h buffer (this half only): qs1's evictions wait on
                    # qs0's FF2 reads, which precede them in Tensor order.
                    h_sb = h_pool.tile([P, FT, 512], f8, tag="h", name=f"h{qs}")
                    for fc in range(NFC):
                        g = qs * NFC + fc
                        w1_refill(g + 3)
                        w1c = w1s.pop(g)
                        for fl in range(FCH):
                            f_t = fc * FCH + fl
                            ps = ps_h.tile([P, 512], f32, tag="ps")
                            for dp in range(ET // 2):
                                nc.tensor.matmul(
                                    ps, w1c[:, 2 * dp:2 * dp + 2, fl * P:(fl + 1) * P],
                                    xT[:, 2 * dp:2 * dp + 2, qs * 512:(qs + 1) * 512],
                                    start=(dp == 0), stop=(dp == ET // 2 - 1),
                                    perf_mode=DR)
                            nc.scalar.activation(
                                out=h_sb[:, f_t, :], in_=ps,
                                func=AF.Relu, bias=b1_sb[:, f_t:f_t + 1],
                                scale=1.0 / W1_SCALE)
                    # FF2 + LN2 per 128-row block of this half
                    for q_t in range(qs * (ET // 2), (qs + 1) * (ET // 2)):
                        ql = (q_t - qs * (ET // 2)) * P
                        asum = lnp.tile([P, 4], f32, tag="asumD", name=f"asumD{q_t}")
                        for eo in range(2):
                            ps = ps_f.tile([P, 512], f32, tag="ps")
                            for fc in range(NFC):
                                for fp_ in range(FCH // 2):
                                    ft0 = fc * FCH + 2 * fp_
                                    nc.tensor.matmul(
                                        ps, h_sb[:, ft0:ft0 + 2, ql:ql + P],
                                        w2cs[fc][:, 2 * fp_:2 * fp_ + 2, eo * 512:(eo + 1) * 512],
                                        start=(fc == 0 and fp_ == 0),
                                        stop=(fc == NFC - 1 and fp_ == FCH // 2 - 1),
                                        perf_mode=DR)
                            dst = x_sb[:, q_t, eo * 512:(eo + 1) * 512]
                            # x += ps/W2_SCALE; same op emits sum(x) per half
                            nc.vector.scalar_tensor_tensor(
                                out=dst, in0=ps, scalar=inv_w2s, in1=dst,
                                op0=MULT, op1=ADD, accum_out=asum[:, eo:eo + 1])
                            sqscr = sqs2_pool.tile([P, 512], f32, tag="sqs")
                            nc.scalar.activation(out=sqscr, in_=dst, func=AF.Square,
                                                 accum_out=asum[:, 2 + eo:3 + eo])
                        mu, rstd, _ = ln_from_asum(asum, "D")
                        # fused affine ((x-mu)*g2)*rstd + be2 as Vector STT
                        # pairs; finer slices on the last rows pipeline the
                        # exposed post-matmul chain against the store DMAs
                        nsl = 4 if q_t == ET - 1 else 2
                        w = E // nsl
                        for sl in range(nsl):
                            hs = slice(sl * w, (sl + 1) * w)
                            row = x_sb[:, q_t, hs]
                            nc.vector.scalar_tensor_tensor(
                                out=row, in0=row, scalar=mu, in1=g2_bc[:, hs],
                                op0=SUB, op1=MULT)
                            nc.vector.scalar_tensor_tensor(
                                out=row, in0=row, scalar=rstd, in1=be2_bc[:, hs],
                                op0=MULT, op1=ADD)
                            # alternate store queues: Scalar is idle here and
                            # each DMA instruction costs ~650ns of issue time
                            eng = nc.scalar if sl % 2 else nc.sync
                            eng.dma_start(
                                out=out[q_t * P:(q_t + 1) * P, hs], in_=row)
        finally:
            late.close()

    nc.compile()
    return nc


_NC_CACHE = None


def _pretile(inputs):
    """Host-side weight folds + re-layouts so every DMA is contiguous per
    partition. W1/W2 are scaled and quantized to fp8 e4m3 (uint8 bytes)."""
    import ml_dtypes
    e4m3 = ml_dtypes.float8_e4m3fn
    bf = ml_dtypes.bfloat16
    c = np.ascontiguousarray
    Wq = np.asarray(inputs["Wq"], np.float32)
    Wk = np.asarray(inputs["Wk"], np.float32)
    Wv = np.asarray(inputs["Wv"], np.float32)
    Wo = np.asarray(inputs["Wo"], np.float32)
    # the single-head unscaled-attention folds: scores = srcq@Wp@src^T,
    # attn-out = P_norm@src@Wvo  (see module docstring)
    Wp = Wq @ Wk.T
    Wvo = Wv @ Wo
    bqk = np.asarray(inputs["bq"], np.float32) @ Wk.T
    # fold the LN1 affine into the first FF layer (exact):
    #   (g1*x^ + be1) @ W1 + b1  ==  x^ @ (diag(g1) @ W1) + (b1 + be1 @ W1)
    W1raw = np.asarray(inputs["W1"], np.float32)
    g1v = np.asarray(inputs["g1"], np.float32)
    be1v = np.asarray(inputs["be1"], np.float32)
    W1 = (g1v[:, None] * W1raw) * W1_SCALE
    b1f = np.asarray(inputs["b1"], np.float32) + be1v @ W1raw
    W2 = np.asarray(inputs["W2"], np.float32) * W2_SCALE
    d = {
        "Wpt": c(Wp.reshape(ET, P, ET, P).transpose(1, 2, 0, 3).astype(bf)),
        "Wvot": c(Wvo.reshape(ET, P, E).transpose(1, 0, 2).astype(bf)),
        "W1t": c(W1.reshape(ET, P, NFC, 512).transpose(1, 2, 0, 3)
                 .astype(e4m3)).view(np.uint8),
        "W2t": c(W2.reshape(NFC, FCH, P, E).transpose(2, 0, 1, 3)
                 .astype(e4m3)).view(np.uint8),
        "bqb1": c(np.concatenate(
            [bqk.reshape(ET, P).T, b1f.reshape(FT, P).T], axis=1)),
        "beb2": c(be1v + np.asarray(inputs["b2"], np.float32)),
        "g1": c(g1v),
    }
    for n in ["g2", "be2"]:
        d[n] = c(np.asarray(inputs[n], np.float32))
    return d


def make_in_maps(inputs):
    import ml_dtypes
    src = np.ascontiguousarray(np.asarray(inputs["src"], dtype=np.float32))
    # residual offset: out-projection bias + bv routed through Wo
    # (attention weights sum to 1, so  softmax(S) @ (V0 + bv) @ Wo + bo
    #  == softmax(S) @ V0 @ Wo + (bv @ Wo + bo))
    bo = (np.asarray(inputs["bo"], np.float32)
          + np.asarray(inputs["bv"], np.float32)
          @ np.asarray(inputs["Wo"], np.float32))
    shared = _pretile(inputs)

    in_maps = []
    for core in range(8):
        b, h = core // 2, core % 2
        src_b = src[b]                        # [2048, 1024]
        # permute context so this core's query half is rows/cols 0..1023
        perm = np.concatenate([src_b[h * NQ:(h + 1) * NQ, :],
                               src_b[(1 - h) * NQ:(2 - h) * NQ, :]])
        srcT = perm.T                         # [1024, 2048]
        # [p, c, t, k] = srcT[t*128+p, c*512+k]; bf16 on the wire
        srcTt = np.ascontiguousarray(
            srcT.reshape(ET, P, NCH, KCH).transpose(1, 2, 0, 3)
            .astype(ml_dtypes.bfloat16))
        # k-major copy for the P@src accumulation: [p, ck, e]
        srcKt = np.ascontiguousarray(
            perm.reshape(NCH * KT, P, E).transpose(1, 0, 2)
            .astype(ml_dtypes.bfloat16))
        # bo' (the folded output bias) rides the residual here; bf16 on the
        # wire (0.4% on the residual, small vs the fp8-FF error), pre-tiled
        # partition-major [p, q_t, e] for batched DMA
        srcq = np.ascontiguousarray(
            (src_b[h * NQ:(h + 1) * NQ, :] + bo)
            .reshape(ET, P, E).transpose(1, 0, 2).astype(ml_dtypes.bfloat16))
        in_maps.append({"srcTt": srcTt, "srcKt": srcKt, "srcq": srcq, **shared})
    return in_maps


def gather_out(results):
    out = np.empty((4, S, E), np.float32)
    for core in range(8):
        b, h = core // 2, core % 2
        out[b, h * NQ:(h + 1) * NQ, :] = results[core]["out"]
    return out


def kernel(**inputs):
    global _NC_CACHE
    from concourse.bass_utils import run_bass_kernel_spmd

    in_maps = make_in_maps(inputs)
    if _NC_CACHE is None:
        _NC_CACHE = build_program()
    res = run_bass_kernel_spmd(_NC_CACHE, in_maps, list(range(8)))
    return gather_out(res.results)


if __name__ == "__main__":
    nc = build_program()
    print("build + compile OK")


# revision 49
# speedup vs baseline: 1.0063x; 1.0063x over previous
"""Trainium2 Bass kernel for a single-head transformer layer (dense_transformer).

Reference math (fp32, unscaled single-head attention):
    Q = src@Wq+bq; K = src@Wk+bk; V = src@Wv+bv
    attn = softmax(Q@K^T) @ V @ Wo + bo
    x  = LN(src + attn)*g1 + be1
    out = LN(x + relu(x@W1+b1)@W2 + b2)*g2 + be2

Sharding: 8 cores = 4 batches x 2 query halves, no collectives. Each core
computes its 1024 query rows against the full 2048-token context of its
batch. srcT is column-PERMUTED per core so the core's own query half
occupies columns 0..1023 (attention is permutation-invariant over context
order).

Weight-fold restructure (the big win vs a direct mapping): because the
attention is single-head and unscaled,
    scores = (srcq@Wq + bq) @ (src@Wk)^T = srcq @ (Wq@Wk^T) @ src^T + bqk@src^T
    attn@Wo = softmax(scores) @ src @ (Wv@Wo)
so the host precomputes Wp = Wq@Wk^T and Wvo = Wv@Wo once (1024^2 each)
and the device never projects K or V at all:
    qT = Wp^T-contract(srcq)  [2.1 GF]   (replaces Q proj + K proj: 6.4 GF)
    S  = qT vs srcT directly  [4.3 GF]
    aT = src^T-contract(P)    [4.3 GF]   (replaces V proj + P@V: 8.6 GF)
    x  = aT^T @ Wvo           [2.1 GF]   (absorbs the O proj)
Per-core Tensor work drops from ~30 GF-bf16-equivalent to ~21.5.

Attention path runs bf16 (scores from bf16 Wp/src; P stored bf16 --
softmax renormalization is computed from the same bf16 P, so the
quantization cancels in the weighted average). The feedforward runs in
fp8 e4m3 with perf_mode=DoubleRow: W1/W2 are quantized host-side with
x16/x32 scaling; exact inverse scales applied in the relu epilogue and
FF2 eviction. All DRAM tensors are host PRE-TILED so every DMA is a
contiguous 2-32KB run per partition.

Exact host-side folds (no accuracy cost): bk dropped (softmax-invariant),
bv -> residual as bv@Wo, bo -> srcq, bq -> bqk = bq@Wk^T (per-partition
bias on qT), LN1 affine -> W1' = diag(g1)@W1, b1' = b1 + be1@W1, and
be1+b2 -> one residual vector. The kernel keeps only normalized x-hat
(bf16) out of LN1; the residual base x-hat*g1 + (be1+b2) is rebuilt in
phase D's Vector slack.

Per-core phases (Tensor stream is never intentionally idle):
  A: qT[e,q] = Wp.T-contract @ srcT_{0,1} + bqk   (Scalar Identity evict)
  B: per 512-chunk c: S^T = srcT_c-contract @ qT; pc = exp(S^T) (bf16);
     sums += 1^T @ pc (PSUM across all chunks); aT += srcK_c^T @ pc
     (SBUF bf16 accumulation). Then sums -> partitions via K=1 matmuls,
     reciprocal -> rsum.
  C: x = (aT^T @ Wvo)*rsum + (src+bo'); LN1 stats ride the eviction
     (accum_out + Scalar Square); x-hat = Scalar Identity(scale=rstd,
     bias=-mu*rstd) -> bf16; PE-transpose (bf16, 1 cyc/row) -> xT fp8.
     Transposes lag one row so the Tensor never waits on the LN chain.
  D: xbase = x-hat*g1 + (be1+b2); per query half: FF1 (fp8 DR,
     relu*1/16+b1' -> h fp8), then per 128-row: FF2 (fp8 DR, 16-matmul
     PSUM accumulation), eviction x = xbase + ps/32 emits LN2 sums via
     accum_out + Scalar Square; LN2 normalize on Scalar, *g2+be2 on
     Vector, store. Row epilogues hide under the next row's matmuls.
"""

import numpy as np
from contextlib import ExitStack

import concourse.bacc as bacc
import concourse.tile as tile
from concourse import mybir
from concourse.masks import make_identity

P = 128
E = 1024          # embed
F = 4096          # dff
S = 2048          # context length per batch
NQ = 1024         # query rows per core
ET = E // P       # 8
FT = F // P       # 32
QS = NQ // 512    # 2 query slices of 512
KCH = 512         # k-chunk size
NCH = S // KCH    # 4 chunks
KT = KCH // P     # 4 k-tiles per chunk
FCH = 4           # f-tiles per FF chunk (512 f-columns)
NFC = FT // FCH   # 8 FF chunks
W1_SCALE = 16.0   # host premultiplier on W1 before e4m3 cast
W2_SCALE = 32.0   # host premultiplier on W2 before e4m3 cast
f32 = mybir.dt.float32
bf16 = mybir.dt.bfloat16
f8 = mybir.dt.float8e4
u8 = mybir.dt.uint8
DR = mybir.MatmulPerfMode.DoubleRow
EPS = 1e-5

SUB = mybir.AluOpType.subtract
MULT = mybir.AluOpType.mult
ADD = mybir.AluOpType.add
AF = mybir.ActivationFunctionType


def _f8(ap):
    """uint8 DRAM bytes -> fp8e4m3 view (numpy has no fp8 dtype)."""
    return ap.bitcast(f8)


def build_program():
    nc = bacc.Bacc("TRN2", target_bir_lowering=False, debug=False, num_devices=8)

    srcTt = nc.dram_tensor("srcTt", [P, NCH, ET, KCH], bf16, kind="ExternalInput").ap()
    srcKt = nc.dram_tensor("srcKt", [P, NCH * KT, E], bf16, kind="ExternalInput").ap()
    srcq = nc.dram_tensor("srcq", [P, ET, E], bf16, kind="ExternalInput").ap()
    Wpt = nc.dram_tensor("Wpt", [P, ET, ET, P], bf16, kind="ExternalInput").ap()
    Wvot = nc.dram_tensor("Wvot", [P, ET, E], bf16, kind="ExternalInput").ap()
    W1t = nc.dram_tensor("W1t", [P, NFC, ET, 512], u8, kind="ExternalInput").ap()
    W2t = nc.dram_tensor("W2t", [P, NFC, FCH, E], u8, kind="ExternalInput").ap()
    # bqk | b1' pre-tiled host-side to [P, ET+FT]: element i at [i%128, i//128]
    bqb1 = nc.dram_tensor("bqb1", [P, ET + FT], f32, kind="ExternalInput").ap()
    beb2 = nc.dram_tensor("beb2", [E], f32, kind="ExternalInput").ap()
    g1 = nc.dram_tensor("g1", [E], f32, kind="ExternalInput").ap()
    g2 = nc.dram_tensor("g2", [E], f32, kind="ExternalInput").ap()
    be2 = nc.dram_tensor("be2", [E], f32, kind="ExternalInput").ap()
    out = nc.dram_tensor("out", [NQ, E], f32, kind="ExternalOutput").ap()

    with tile.TileContext(nc) as tc, ExitStack() as ctx:
        consts = ctx.enter_context(tc.tile_pool(name="consts", bufs=1))
        lnp = ctx.enter_context(tc.tile_pool(name="lnp", bufs=6))
        bcp = ctx.enter_context(tc.tile_pool(name="bcp", bufs=1))
        # long-lived tensors allocated BEFORE the phase-A/B pools so their
        # DMAs never alias (and never wait on) the attention working set
        aT_pool = ctx.enter_context(tc.tile_pool(name="aT_pool", bufs=1))
        wvo_pool = ctx.enter_context(tc.tile_pool(name="wvo", bufs=1))
        # 4-deep W1 ring: chunks stream through twice (once per query
        # half) instead of sitting fully resident -- saves 16KB/partition
        w1_pool = ctx.enter_context(tc.tile_pool(name="w1p", bufs=4))
        sq_pool = ctx.enter_context(tc.tile_pool(name="sqp", bufs=1))

        aT = aT_pool.tile([P, ET, NQ], bf16)
        wvo_sb = wvo_pool.tile([P, ET, E], bf16)
        # residual src+bo' prefetched bf16 during attention: phase C must
        # not wait on DMAs that queue behind the W2 prefetch burst
        sq_sb = sq_pool.tile([P, ET, E], bf16)

        bias_sb = consts.tile([P, ET + FT], f32)
        nc.sync.dma_start(out=bias_sb, in_=bqb1)
        bqk_sb = bias_sb[:, 0:ET]
        b1_sb = bias_sb[:, ET:]
        ones_bf = consts.tile([P, 1], bf16)
        nc.vector.memset(ones_bf, 1.0)
        one_sp = consts.tile([1, 1], f32)
        nc.vector.memset(one_sp, 1.0)
        eps_sb = consts.tile([P, 1], f32)
        nc.vector.memset(eps_sb, EPS)
        inv_w2s = consts.tile([P, 1], f32)
        nc.vector.memset(inv_w2s, 1.0 / W2_SCALE)
        sums_sb = consts.tile([1, NQ], f32)
        rsum = consts.tile([P, ET], f32)
        ident_bf = consts.tile([P, P], bf16)
        make_identity(nc, ident_bf)

        def bcast(vec, n, pool, eng=None):
            t = pool.tile([P, n], f32, tag=f"bc_{vec.tensor.name}")
            (eng or nc.sync).dma_start(out=t, in_=vec.partition_broadcast(P))
            return t

        def ln_from_asum(asum, tag, want_nmr=False):
            """mu/rstd (and optionally nmr = -mu*rstd for the Scalar-side
            normalize) from accumulated [s_x(lo), s_x(hi), s_x2(lo),
            s_x2(hi)]."""
            mu = lnp.tile([P, 1], f32, tag=f"mu{tag}")
            nc.vector.tensor_add(out=mu, in0=asum[:, 0:1], in1=asum[:, 1:2])
            nc.vector.tensor_scalar_mul(out=mu, in0=mu, scalar1=1.0 / E)
            s23 = lnp.tile([P, 1], f32, tag=f"s23{tag}")
            nc.vector.tensor_add(out=s23, in0=asum[:, 2:3], in1=asum[:, 3:4])
            mu2 = lnp.tile([P, 1], f32, tag=f"mu2{tag}")
            nc.vector.tensor_mul(out=mu2, in0=mu, in1=mu)
            vr = lnp.tile([P, 1], f32, tag=f"vr{tag}")
            nc.vector.tensor_scalar(out=vr, in0=s23, scalar1=1.0 / E,
                                    scalar2=mu2, op0=MULT, op1=SUB)
            rstd = lnp.tile([P, 1], f32, tag=f"rstd{tag}")
            nc.scalar.activation(out=rstd, in_=vr, func=AF.Sqrt,
                                 bias=eps_sb, scale=1.0)
            nc.vector.reciprocal(out=rstd, in_=rstd)
            if not want_nmr:
                return mu, rstd, None
            nmr = lnp.tile([P, 1], f32, tag=f"nmr{tag}")
            nc.vector.tensor_scalar(out=nmr, in0=mu, scalar1=rstd,
                                    scalar2=-1.0, op0=MULT, op1=MULT)
            return mu, rstd, nmr

        with ExitStack() as phAB:
            qT_pool = phAB.enter_context(tc.tile_pool(name="qT_pool", bufs=1))
            st_pool = phAB.enter_context(tc.tile_pool(name="stp", bufs=2))
            sta_pool = phAB.enter_context(tc.tile_pool(name="stap", bufs=2))
            stb_pool = phAB.enter_context(tc.tile_pool(name="stbp", bufs=2))
            sk_pool = phAB.enter_context(tc.tile_pool(name="skp", bufs=1))
            wp_pool = phAB.enter_context(tc.tile_pool(name="wpp", bufs=1))
            pc_pool = phAB.enter_context(tc.tile_pool(name="pcp", bufs=2))
            ps_s = phAB.enter_context(tc.tile_pool(name="ps_s", bufs=3, space="PSUM"))
            ps_a = phAB.enter_context(tc.tile_pool(name="ps_a", bufs=3, space="PSUM"))
            ps_sum = phAB.enter_context(tc.tile_pool(name="ps_sum", bufs=1, space="PSUM"))

            qT = qT_pool.tile([P, ET, NQ], bf16)
            sk = sk_pool.tile([P, NCH * KT, E], bf16)

            sums = []
            for qs in range(QS):
                sums.append(ps_sum.tile([1, 512], f32, tag=f"sums{qs}",
                                        name=f"sums{qs}"))

            sts, st_a, st_b = {}, {}, {}

            def st_load(cc, eng=None):
                eng = eng or nc.sync
                st = st_pool.tile([P, ET, KCH], bf16, tag="st", name=f"st{cc}")
                eng.dma_start(out=st, in_=srcTt[:, cc])
                sts[cc] = st

            def st_ap(cc, d_t):
                if cc in st_a:
                    return st_a[cc] if d_t == 0 else st_b[cc][:, d_t - 1, :]
                return sts[cc][:, d_t, :]

            def sk_load(cc, eng):
                eng.dma_start(out=sk[:, cc * KT:(cc + 1) * KT, :],
                              in_=srcKt[:, cc * KT:(cc + 1) * KT, :])

            # -- startup-critical DMAs (Sync queue, enqueued at t=0): ONLY
            # what phases A and early-B need, BATCHED into few instructions
            # (each DMA_DIRECT2D costs ~650ns of Sync-engine issue time; 33
            # separate descriptors serialized the whole startup window),
            # split into tiles sized to the consumption order. Everything
            # else enqueues from the Scalar engine's DGE queue at staged
            # points below, so the startup window isn't fair-shared away to
            # prefetches that aren't needed until t+100us.
            # dual-queue startup: Sync and Scalar DGE queues enqueue in
            # parallel (separate DMA engine groups), halving both the
            # ~650ns/instruction issue serialization and per-queue load
            wp0 = wp_pool.tile([P, ET, P], bf16, tag="wp0")
            wp1 = wp_pool.tile([P, ET, P], bf16, tag="wp1")
            wp23 = wp_pool.tile([P, 2, ET, P], bf16, tag="wp23")
            wp47 = wp_pool.tile([P, 4, ET, P], bf16, tag="wp47")
            st0a = sta_pool.tile([P, KCH], bf16, tag="sta", name="st0a")
            st0b = stb_pool.tile([P, ET - 1, KCH], bf16, tag="stb", name="st0b")
            st1a = sta_pool.tile([P, KCH], bf16, tag="sta", name="st1a")
            st1b = stb_pool.tile([P, ET - 1, KCH], bf16, tag="stb", name="st1b")
            st_a[0], st_b[0], st_a[1], st_b[1] = st0a, st0b, st1a, st1b
            nc.sync.dma_start(out=wp0, in_=Wpt[:, 0])
            nc.scalar.dma_start(out=st0b, in_=srcTt[:, 0, 1:, :])
            nc.sync.dma_start(out=st0a, in_=srcTt[:, 0, 0, :])
            nc.scalar.dma_start(out=st1a, in_=srcTt[:, 1, 0, :])
            nc.sync.dma_start(out=wp1, in_=Wpt[:, 1])
            nc.scalar.dma_start(out=st1b, in_=srcTt[:, 1, 1:, :])
            nc.sync.dma_start(out=wp23, in_=Wpt[:, 2:4])
            nc.sync.dma_start(out=wp47, in_=Wpt[:, 4:])
            nc.scalar.dma_start(out=sk[:, 0:KT, :], in_=srcKt[:, 0:KT, :])
            wp_ts = ([wp0, wp1] + [wp23[:, i] for i in range(2)]
                     + [wp47[:, i] for i in range(4)])

            # ---------------- Phase A: qT = Wp-contract(srcq) ----------------
            # (e_t=0, qs=1) deferred one iteration so the first matmuls only
            # need st0 -- st1 is still streaming in at that point.
            def qproj(e_t, qs):
                ps = ps_s.tile([P, 512], f32, tag="ps")
                for d_t in range(ET):
                    nc.tensor.matmul(ps, wp_ts[e_t][:, d_t, :], st_ap(qs, d_t),
                                     start=(d_t == 0), stop=(d_t == ET - 1))
                nc.scalar.activation(out=qT[:, e_t, qs * 512:(qs + 1) * 512],
                                     in_=ps, func=AF.Identity,
                                     bias=bqk_sb[:, e_t:e_t + 1], scale=1.0)

            for e_t in range(ET):
                qproj(e_t, 0)
                if e_t == 1:
                    qproj(0, 1)
                if e_t >= 1:
                    qproj(e_t, 1)

            # deferred prefetch, stage 1 (enqueues after A's evictions)
            st_load(2, eng=nc.scalar)
            sk_load(1, nc.scalar)

            # ---------------- Phase B: scores -> exp -> sums/aT --------------
            for cc in range(NCH):
                pc = pc_pool.tile([P, KT, NQ], bf16, tag="pc", name=f"pc{cc}")
                for qs in range(QS):
                    for kt in range(KT):
                        ps = ps_s.tile([P, 512], f32, tag="ps")
                        for d_t in range(ET):
                            nc.tensor.matmul(
                                ps, st_ap(cc, d_t)[:, kt * P:(kt + 1) * P],
                                qT[:, d_t, qs * 512:(qs + 1) * 512],
                                start=(d_t == 0), stop=(d_t == ET - 1))
                        nc.scalar.activation(
                            out=pc[:, kt, qs * 512:(qs + 1) * 512], in_=ps,
                            func=AF.Exp)
                        nc.tensor.matmul(sums[qs], ones_bf,
                                         pc[:, kt, qs * 512:(qs + 1) * 512],
                                         start=(cc == 0 and kt == 0),
                                         stop=(cc == NCH - 1 and kt == KT - 1))
                # deferred prefetch stages 2-4: enqueue from the Scalar
                # queue right after this chunk's exp evictions
                if cc == 0:
                    st_load(3, eng=nc.scalar)
                    sk_load(2, nc.scalar)
                elif cc == 1:
                    sk_load(3, nc.scalar)
                    nc.scalar.dma_start(out=wvo_sb[:, 0:4], in_=Wvot[:, 0:4])
                    nc.scalar.dma_start(out=wvo_sb[:, 4:], in_=Wvot[:, 4:])
                elif cc == 2:
                    nc.scalar.dma_start(out=sq_sb[:, 0:4], in_=srcq[:, 0:4])
                    nc.scalar.dma_start(out=sq_sb[:, 4:], in_=srcq[:, 4:])
                    g1_bc = bcast(g1, E, bcp, nc.scalar)
                    beb2_bc = bcast(beb2, E, bcp, nc.scalar)
                    g2_bc = bcast(g2, E, bcp, nc.scalar)
                    be2_bc = bcast(be2, E, bcp, nc.scalar)
                    w1_pre = []
                    for fc in range(4):
                        w1c = w1_pool.tile([P, ET, 512], f8, tag="w1",
                                           name=f"w1c{fc}")
                        nc.scalar.dma_start(out=w1c, in_=_f8(W1t[:, fc]))
                        w1_pre.append(w1c)
                if cc == NCH - 1:
                    # softmax denominators: spread sums[1, q] across
                    # partitions via K=1 matmuls (1-partition DMAs fail NEFF
                    # load). Hoisted BEFORE the last chunk's aT block so the
                    # rsum chain completes under it and phase C starts hot.
                    for qs in range(QS):
                        nc.vector.tensor_copy(
                            out=sums_sb[:, qs * 512:(qs + 1) * 512],
                            in_=sums[qs])
                    for t in range(ET):
                        pst = ps_a.tile([P, 1], f32, tag="ps", name=f"spread{t}")
                        nc.tensor.matmul(pst, sums_sb[0:1, t * P:(t + 1) * P],
                                         one_sp, start=True, stop=True)
                        nc.vector.tensor_copy(out=rsum[:, t:t + 1], in_=pst)
                    nc.vector.reciprocal(out=rsum, in_=rsum)
                for qs in range(QS):
                    for d_t in range(ET):
                        ps = ps_a.tile([P, 512], f32, tag="ps")
                        for kt in range(KT):
                            nc.tensor.matmul(
                                ps, sk[:, cc * KT + kt, d_t * P:(d_t + 1) * P],
                                pc[:, kt, qs * 512:(qs + 1) * 512],
                                start=(kt == 0), stop=(kt == KT - 1))
                        dst = aT[:, d_t, qs * 512:(qs + 1) * 512]
                        if cc == 0:
                            nc.vector.tensor_copy(out=dst, in_=ps)
                        else:
                            nc.vector.tensor_add(out=dst, in0=dst, in1=ps)

        # late pools: land in the freed attention SBUF; their DMAs start
        # as soon as the aliased phase-A/B tiles' last reads complete
        late = ExitStack()
        try:
            x_pool = late.enter_context(tc.tile_pool(name="x_pool", bufs=1))
            xb_pool = late.enter_context(tc.tile_pool(name="xb_pool", bufs=1))
            xT_pool = late.enter_context(tc.tile_pool(name="xT_pool", bufs=1))
            w2_pool = late.enter_context(tc.tile_pool(name="w2p", bufs=NFC))
            x_sb = x_pool.tile([P, ET, E], f32)    # [q(8x128), e] residual base
            xb16 = xb_pool.tile([P, ET, E], bf16)  # normalized x-hat
            xT = xT_pool.tile([P, ET, NQ], f8)     # [e, q] fp8 for the FF

            w2cs = []
            for fc in range(NFC):
                w2c = w2_pool.tile([P, FCH, E], f8, tag="w2", name=f"w2c{fc}")
                nc.sync.dma_start(out=w2c, in_=_f8(W2t[:, fc]))
                w2cs.append(w2c)

            # ---------- Phase C: x = aT^T@Wvo + residual, LN1, transpose -----
            with ExitStack() as ph3:
                sqs_pool = ph3.enter_context(tc.tile_pool(name="sqs", bufs=2))
                ps_o = ph3.enter_context(tc.tile_pool(name="ps_o", bufs=3, space="PSUM"))
                ps_t = ph3.enter_context(tc.tile_pool(name="ps_t", bufs=2, space="PSUM"))

                def transposes(q_t):
                    """xb16 row q_t -> xT (bf16 PE transposes, fp8 evict)."""
                    for ep in range(ET // 2):
                        pst = ps_t.tile([P, 2, P], bf16, tag="pst")
                        for j in range(2):
                            nc.tensor.transpose(
                                pst[:, j, :],
                                xb16[:, q_t, (2 * ep + j) * P:(2 * ep + j + 1) * P],
                                ident_bf)
                        nc.vector.tensor_copy(
                            out=xT[:, 2 * ep:2 * ep + 2, q_t * P:(q_t + 1) * P],
                            in_=pst)

                for q_t in range(ET):
                    asum = lnp.tile([P, 4], f32, tag="asumC", name=f"asumC{q_t}")
                    for eo in range(2):
                        ps = ps_o.tile([P, 512], f32, tag="ps")
                        for d_t in range(ET):
                            nc.tensor.matmul(ps, aT[:, d_t, q_t * P:(q_t + 1) * P],
                                             wvo_sb[:, d_t, eo * 512:(eo + 1) * 512],
                                             start=(d_t == 0), stop=(d_t == ET - 1))
                        dst = x_sb[:, q_t, eo * 512:(eo + 1) * 512]
                        # x = O*rsum + (src+bo'), one fused Vector op; emits
                        # sum(x) per half for LN1 (bo' = bo + bv@Wo, on host)
                        nc.vector.scalar_tensor_tensor(
                            out=dst, in0=ps, scalar=rsum[:, q_t:q_t + 1],
                            in1=sq_sb[:, q_t, eo * 512:(eo + 1) * 512],
                            op0=MULT, op1=ADD, accum_out=asum[:, eo:eo + 1])
                        sqscr = sqs_pool.tile([P, 512], f32, tag="sqs")
                        nc.scalar.activation(out=sqscr, in_=dst, func=AF.Square,
                                             accum_out=asum[:, 2 + eo:3 + eo])
                    _, rstd, nmr = ln_from_asum(asum, "C", want_nmr=True)
                    # normalize-only LN1 (affine folded into W1/b1'), bf16
                    nc.scalar.activation(out=xb16[:, q_t, :], in_=x_sb[:, q_t, :],
                                         func=AF.Identity, bias=nmr, scale=rstd)
                    # transposes lag one row so Tensor never waits on LN1
                    if q_t > 0:
                        transposes(q_t - 1)
                transposes(ET - 1)

            # ---------------- Phase D: feedforward (fp8 DR) + LN2 -----------
            with ExitStack() as ph4:
                h_pool = ph4.enter_context(tc.tile_pool(name="hp", bufs=1))
                sqs2_pool = ph4.enter_context(tc.tile_pool(name="sqs2", bufs=2))
                ps_h = ph4.enter_context(tc.tile_pool(name="ps_h", bufs=2, space="PSUM"))
                ps_f = ph4.enter_context(tc.tile_pool(name="ps_f", bufs=2, space="PSUM"))

                # rebuild the residual base x-hat*g1 + (be1+b2) in FF1's
                # Vector slack (x_sb rows are free post-normalize)
                for q_t in range(ET):
                    row = x_sb[:, q_t, :]
                    nc.vector.tensor_mul(out=row, in0=xb16[:, q_t, :], in1=g1_bc)
                    nc.vector.tensor_add(out=row, in0=row, in1=beb2_bc)

                # W1 ring refills on the Scalar queue (lookahead 2): the
                # Sync queue head-of-line blocks behind LN2-gated output
                # stores, which would stall FF1(qs1)'s weight loads
                w1s = {g: w1_pre[g] for g in range(4)}

                def w1_refill(g):
                    if g > 15 or g in w1s:
                        return
                    t = w1_pool.tile([P, ET, 512], f8, tag="w1", name=f"w1g{g}")
                    nc.scalar.dma_start(out=t, in_=_f8(W1t[:, g % NFC]))
                    w1s[g] = t

                for qs in range(QS):
                    # FF1 for this query half: h = relu(ps/16 + b1') fp8.
                    # One h buffer (this half only): qs1's evictions wait on
                    # qs0's FF2 reads, which precede them in Tensor order.
                    h_sb = h_pool.tile([P, FT, 512], f8, tag="h", name=f"h{qs}")
                    for fc in range(NFC):
                        g = qs * NFC + fc
                        w1_refill(g + 2)
                        w1c = w1s.pop(g)
                        for fl in range(FCH):
                            f_t = fc * FCH + fl
                            ps = ps_h.tile([P, 512], f32, tag="ps")
                            for dp in range(ET // 2):
                                nc.tensor.matmul(
                                    ps, w1c[:, 2 * dp:2 * dp + 2, fl * P:(fl + 1) * P],
                                    xT[:, 2 * dp:2 * dp + 2, qs * 512:(qs + 1) * 512],
                                    start=(dp == 0), stop=(dp == ET // 2 - 1),
                                    perf_mode=DR)
                            nc.scalar.activation(
                                out=h_sb[:, f_t, :], in_=ps,
                                func=AF.Relu, bias=b1_sb[:, f_t:f_t + 1],
                                scale=1.0 / W1_SCALE)
                    # FF2 + LN2 per 128-row block of this half
                    for q_t in range(qs * (ET // 2), (qs + 1) * (ET // 2)):
                        ql = (q_t - qs * (ET // 2)) * P
                        asum = lnp.tile([P, 4], f32, tag="asumD", name=f"asumD{q_t}")
                        for eo in range(2):
                            ps = ps_f.tile([P, 512], f32, tag="ps")
                            for fc in range(NFC):
                                for fp_ in range(FCH // 2):
                                    ft0 = fc * FCH + 2 * fp_
                                    nc.tensor.matmul(
                                        ps, h_sb[:, ft0:ft0 + 2, ql:ql + P],
                                        w2cs[fc][:, 2 * fp_:2 * fp_ + 2, eo * 512:(eo + 1) * 512],
                                        start=(fc == 0 and fp_ == 0),
                                        stop=(fc == NFC - 1 and fp_ == FCH // 2 - 1),
                                        perf_mode=DR)
                            dst = x_sb[:, q_t, eo * 512:(eo + 1) * 512]
                            # x += ps/W2_SCALE; same op emits sum(x) per half
                            nc.vector.scalar_tensor_tensor(
                                out=dst, in0=ps, scalar=inv_w2s, in1=dst,
                                op0=MULT, op1=ADD, accum_out=asum[:, eo:eo + 1])
                            sqscr = sqs2_pool.tile([P, 512], f32, tag="sqs")
                            nc.scalar.activation(out=sqscr, in_=dst, func=AF.Square,
                                                 accum_out=asum[:, 2 + eo:3 + eo])
                        mu, rstd, _ = ln_from_asum(asum, "D")
                        # fused affine ((x-mu)*g2)*rstd + be2 as Vector STT
                        # pairs; finer slices on the last rows pipeline the
                        # exposed post-matmul chain against the store DMAs
                        nsl = 4 if q_t == ET - 1 else 2
                        w = E // nsl
                        for sl in range(nsl):
                            hs = slice(sl * w, (sl + 1) * w)
                            row = x_sb[:, q_t, hs]
                            nc.vector.scalar_tensor_tensor(
                                out=row, in0=row, scalar=mu, in1=g2_bc[:, hs],
                                op0=SUB, op1=MULT)
                            nc.vector.scalar_tensor_tensor(
                                out=row, in0=row, scalar=rstd, in1=be2_bc[:, hs],
                                op0=MULT, op1=ADD)
                            # alternate store queues: Scalar is idle here and
                            # each DMA instruction costs ~650ns of issue time
                            eng = nc.scalar if sl % 2 else nc.sync
                            eng.dma_start(
                                out=out[q_t * P:(q_t + 1) * P, hs], in_=row)
        finally:
            late.close()

    nc.compile()
    return nc


_NC_CACHE = None


def _pretile(inputs):
    """Host-side weight folds + re-layouts so every DMA is contiguous per
    partition. W1/W2 are scaled and quantized to fp8 e4m3 (uint8 bytes)."""
    import ml_dtypes
    e4m3 = ml_dtypes.float8_e4m3fn
    bf = ml_dtypes.bfloat16
    c = np.ascontiguousarray
    Wq = np.asarray(inputs["Wq"], np.float32)
    Wk = np.asarray(inputs["Wk"], np.float32)
    Wv = np.asarray(inputs["Wv"], np.float32)
    Wo = np.asarray(inputs["Wo"], np.float32)
    # the single-head unscaled-attention folds: scores = srcq@Wp@src^T,
    # attn-out = P_norm@src@Wvo  (see module docstring)
    Wp = Wq @ Wk.T
    Wvo = Wv @ Wo
    bqk = np.asarray(inputs["bq"], np.float32) @ Wk.T
    # fold the LN1 affine into the first FF layer (exact):
    #   (g1*x^ + be1) @ W1 + b1  ==  x^ @ (diag(g1) @ W1) + (b1 + be1 @ W1)
    W1raw = np.asarray(inputs["W1"], np.float32)
    g1v = np.asarray(inputs["g1"], np.float32)
    be1v = np.asarray(inputs["be1"], np.float32)
    W1 = (g1v[:, None] * W1raw) * W1_SCALE
    b1f = np.asarray(inputs["b1"], np.float32) + be1v @ W1raw
    W2 = np.asarray(inputs["W2"], np.float32) * W2_SCALE
    d = {
        "Wpt": c(Wp.reshape(ET, P, ET, P).transpose(1, 2, 0, 3).astype(bf)),
        "Wvot": c(Wvo.reshape(ET, P, E).transpose(1, 0, 2).astype(bf)),
        "W1t": c(W1.reshape(ET, P, NFC, 512).transpose(1, 2, 0, 3)
                 .astype(e4m3)).view(np.uint8),
        "W2t": c(W2.reshape(NFC, FCH, P, E).transpose(2, 0, 1, 3)
                 .astype(e4m3)).view(np.uint8),
        "bqb1": c(np.concatenate(
            [bqk.reshape(ET, P).T, b1f.reshape(FT, P).T], axis=1)),
        "beb2": c(be1v + np.asarray(inputs["b2"], np.float32)),
        "g1": c(g1v),
    }
    for n in ["g2", "be2"]:
        d[n] = c(np.asarray(inputs[n], np.float32))
    return d


def make_in_maps(inputs):
    import ml_dtypes
    src = np.ascontiguousarray(np.asarray(inputs["src"], dtype=np.float32))
    # residual offset: out-projection bias + bv routed through Wo
    # (attention weights sum to 1, so  softmax(S) @ (V0 + bv) @ Wo + bo
    #  == softmax(S) @ V0 @ Wo + (bv @ Wo + bo))
    bo = (np.asarray(inputs["bo"], np.float32)
          + np.asarray(inputs["bv"], np.float32)
          @ np.asarray(inputs["Wo"], np.float32))
    shared = _pretile(inputs)

    in_maps = []
    for core in range(8):
        b, h = core // 2, core % 2
        src_b = src[b]                        # [2048, 1024]
        # permute context so this core's query half is rows/cols 0..1023
        perm = np.concatenate([src_b[h * NQ:(h + 1) * NQ, :],
                               src_b[(1 - h) * NQ:(2 - h) * NQ, :]])
        srcT = perm.T                         # [1024, 2048]
        # [p, c, t, k] = srcT[t*128+p, c*512+k]; bf16 on the wire
        srcTt = np.ascontiguousarray(
            srcT.reshape(ET, P, NCH, KCH).transpose(1, 2, 0, 3)
            .astype(ml_dtypes.bfloat16))
        # k-major copy for the P@src accumulation: [p, ck, e]
        srcKt = np.ascontiguousarray(
            perm.reshape(NCH * KT, P, E).transpose(1, 0, 2)
            .astype(ml_dtypes.bfloat16))
        # bo' (the folded output bias) rides the residual here; bf16 on the
        # wire (0.4% on the residual, small vs the fp8-FF error), pre-tiled
        # partition-major [p, q_t, e] for batched DMA
        srcq = np.ascontiguousarray(
            (src_b[h * NQ:(h + 1) * NQ, :] + bo)
            .reshape(ET, P, E).transpose(1, 0, 2).astype(ml_dtypes.bfloat16))
        in_maps.append({"srcTt": srcTt, "srcKt": srcKt, "srcq": srcq, **shared})
    return in_maps


def gather_out(results):
    out = np.empty((4, S, E), np.float32)
    for core in range(8):
        b, h = core // 2, core % 2
        out[b, h * NQ:(h + 1) * NQ, :] = results[core]["out"]
    return out


def kernel(**inputs):
    global _NC_CACHE
    from concourse.bass_utils import run_bass_kernel_spmd

    in_maps = make_in_maps(inputs)
    if _NC_CACHE is None:
        _NC_CACHE = build_program()
    res = run_bass_kernel_spmd(_NC_CACHE, in_maps, list(range(8)))
    return gather_out(res.results)


if __name__ == "__main__":
    nc = build_program()
    print("build + compile OK")


# revision 50
# speedup vs baseline: 1.0152x; 1.0088x over previous
"""Trainium2 Bass kernel for a single-head transformer layer (dense_transformer).

Reference math (fp32, unscaled single-head attention):
    Q = src@Wq+bq; K = src@Wk+bk; V = src@Wv+bv
    attn = softmax(Q@K^T) @ V @ Wo + bo
    x  = LN(src + attn)*g1 + be1
    out = LN(x + relu(x@W1+b1)@W2 + b2)*g2 + be2

Sharding: 8 cores = 4 batches x 2 query halves, no collectives. Each core
computes its 1024 query rows against the full 2048-token context of its
batch. srcT is column-PERMUTED per core so the core's own query half
occupies columns 0..1023 (attention is permutation-invariant over context
order).

Weight-fold restructure (the big win vs a direct mapping): because the
attention is single-head and unscaled,
    scores = (srcq@Wq + bq) @ (src@Wk)^T = srcq @ (Wq@Wk^T) @ src^T + bqk@src^T
    attn@Wo = softmax(scores) @ src @ (Wv@Wo)
so the host precomputes Wp = Wq@Wk^T and Wvo = Wv@Wo once (1024^2 each)
and the device never projects K or V at all:
    qT = Wp^T-contract(srcq)  [2.1 GF]   (replaces Q proj + K proj: 6.4 GF)
    S  = qT vs srcT directly  [4.3 GF]
    aT = src^T-contract(P)    [4.3 GF]   (replaces V proj + P@V: 8.6 GF)
    x  = aT^T @ Wvo           [2.1 GF]   (absorbs the O proj)
Per-core Tensor work drops from ~30 GF-bf16-equivalent to ~21.5.

Attention path runs bf16 (scores from bf16 Wp/src; P stored bf16 --
softmax renormalization is computed from the same bf16 P, so the
quantization cancels in the weighted average). The feedforward runs in
fp8 e4m3 with perf_mode=DoubleRow: W1/W2 are quantized host-side with
x16/x32 scaling; exact inverse scales applied in the relu epilogue and
FF2 eviction. All DRAM tensors are host PRE-TILED so every DMA is a
contiguous 2-32KB run per partition.

Exact host-side folds (no accuracy cost): bk dropped (softmax-invariant),
bv -> residual as bv@Wo, bo -> srcq, bq -> bqk = bq@Wk^T (per-partition
bias on qT), LN1 affine -> W1' = diag(g1)@W1, b1' = b1 + be1@W1, and
be1+b2 -> one residual vector. The kernel keeps only normalized x-hat
(bf16) out of LN1; the residual base x-hat*g1 + (be1+b2) is rebuilt in
phase D's Vector slack.

Per-core phases (Tensor stream is never intentionally idle):
  A: qT[e,q] = Wp.T-contract @ srcT_{0,1} + bqk   (Scalar Identity evict)
  B: per 512-chunk c: S^T = srcT_c-contract @ qT; pc = exp(S^T) (bf16);
     sums += 1^T @ pc (PSUM across all chunks); aT += srcK_c^T @ pc
     (SBUF bf16 accumulation). Then sums -> partitions via K=1 matmuls,
     reciprocal -> rsum.
  C: x = (aT^T @ Wvo)*rsum + (src+bo'); LN1 stats ride the eviction
     (accum_out + Scalar Square); x-hat = Scalar Identity(scale=rstd,
     bias=-mu*rstd) -> bf16; PE-transpose (bf16, 1 cyc/row) -> xT fp8.
     Transposes lag one row so the Tensor never waits on the LN chain.
  D: xbase = x-hat*g1 + (be1+b2); per query half: FF1 (fp8 DR,
     relu*1/16+b1' -> h fp8), then per 128-row: FF2 (fp8 DR, 16-matmul
     PSUM accumulation), eviction x = xbase + ps/32 emits LN2 sums via
     accum_out + Scalar Square; LN2 normalize on Scalar, *g2+be2 on
     Vector, store. Row epilogues hide under the next row's matmuls.
"""

import numpy as np
from contextlib import ExitStack

import concourse.bacc as bacc
import concourse.tile as tile
from concourse import mybir
from concourse.masks import make_identity

P = 128
E = 1024          # embed
F = 4096          # dff
S = 2048          # context length per batch
NQ = 1024         # query rows per core
ET = E // P       # 8
FT = F // P       # 32
QS = NQ // 512    # 2 query slices of 512
KCH = 512         # k-chunk size
NCH = S // KCH    # 4 chunks
KT = KCH // P     # 4 k-tiles per chunk
FCH = 4           # f-tiles per FF chunk (512 f-columns)
NFC = FT // FCH   # 8 FF chunks
W1_SCALE = 16.0   # host premultiplier on W1 before e4m3 cast
W2_SCALE = 32.0   # host premultiplier on W2 before e4m3 cast
f32 = mybir.dt.float32
bf16 = mybir.dt.bfloat16
f8 = mybir.dt.float8e4
u8 = mybir.dt.uint8
DR = mybir.MatmulPerfMode.DoubleRow
EPS = 1e-5

SUB = mybir.AluOpType.subtract
MULT = mybir.AluOpType.mult
ADD = mybir.AluOpType.add
AF = mybir.ActivationFunctionType


def _f8(ap):
    """uint8 DRAM bytes -> fp8e4m3 view (numpy has no fp8 dtype)."""
    return ap.bitcast(f8)


def build_program():
    nc = bacc.Bacc("TRN2", target_bir_lowering=False, debug=False, num_devices=8)

    srcTt = nc.dram_tensor("srcTt", [P, NCH, ET, KCH], bf16, kind="ExternalInput").ap()
    srcKt = nc.dram_tensor("srcKt", [P, NCH * KT, E], bf16, kind="ExternalInput").ap()
    srcq = nc.dram_tensor("srcq", [P, ET, E], bf16, kind="ExternalInput").ap()
    Wpt = nc.dram_tensor("Wpt", [P, ET, ET, P], bf16, kind="ExternalInput").ap()
    Wvot = nc.dram_tensor("Wvot", [P, ET, E], bf16, kind="ExternalInput").ap()
    W1t = nc.dram_tensor("W1t", [P, NFC, ET, 512], u8, kind="ExternalInput").ap()
    W2t = nc.dram_tensor("W2t", [P, NFC, FCH, E], u8, kind="ExternalInput").ap()
    # bqk | b1' pre-tiled host-side to [P, ET+FT]: element i at [i%128, i//128]
    bqb1 = nc.dram_tensor("bqb1", [P, ET + FT], f32, kind="ExternalInput").ap()
    beb2 = nc.dram_tensor("beb2", [E], f32, kind="ExternalInput").ap()
    g1 = nc.dram_tensor("g1", [E], f32, kind="ExternalInput").ap()
    g2 = nc.dram_tensor("g2", [E], f32, kind="ExternalInput").ap()
    be2 = nc.dram_tensor("be2", [E], f32, kind="ExternalInput").ap()
    out = nc.dram_tensor("out", [NQ, E], f32, kind="ExternalOutput").ap()

    with tile.TileContext(nc) as tc, ExitStack() as ctx:
        consts = ctx.enter_context(tc.tile_pool(name="consts", bufs=1))
        lnp = ctx.enter_context(tc.tile_pool(name="lnp", bufs=6))
        bcp = ctx.enter_context(tc.tile_pool(name="bcp", bufs=1))
        # long-lived tensors allocated BEFORE the phase-A/B pools so their
        # DMAs never alias (and never wait on) the attention working set
        aT_pool = ctx.enter_context(tc.tile_pool(name="aT_pool", bufs=1))
        wvo_pool = ctx.enter_context(tc.tile_pool(name="wvo", bufs=1))
        # 4-deep W1 ring: chunks stream through twice (once per query
        # half) instead of sitting fully resident -- saves 16KB/partition
        w1_pool = ctx.enter_context(tc.tile_pool(name="w1p", bufs=4))
        sq_pool = ctx.enter_context(tc.tile_pool(name="sqp", bufs=1))

        aT = aT_pool.tile([P, ET, NQ], bf16)
        wvo_sb = wvo_pool.tile([P, ET, E], bf16)
        # residual src+bo' prefetched bf16 during attention: phase C must
        # not wait on DMAs that queue behind the W2 prefetch burst
        sq_sb = sq_pool.tile([P, ET, E], bf16)

        bias_sb = consts.tile([P, ET + FT], f32)
        nc.sync.dma_start(out=bias_sb, in_=bqb1)
        bqk_sb = bias_sb[:, 0:ET]
        b1_sb = bias_sb[:, ET:]
        ones_bf = consts.tile([P, 1], bf16)
        nc.vector.memset(ones_bf, 1.0)
        one_sp = consts.tile([1, 1], f32)
        nc.vector.memset(one_sp, 1.0)
        eps_sb = consts.tile([P, 1], f32)
        nc.vector.memset(eps_sb, EPS)
        inv_w2s = consts.tile([P, 1], f32)
        nc.vector.memset(inv_w2s, 1.0 / W2_SCALE)
        sums_sb = consts.tile([1, NQ], f32)
        rsum = consts.tile([P, ET], f32)
        ident_bf = consts.tile([P, P], bf16)
        make_identity(nc, ident_bf)

        def bcast(vec, n, pool, eng=None):
            t = pool.tile([P, n], f32, tag=f"bc_{vec.tensor.name}")
            (eng or nc.sync).dma_start(out=t, in_=vec.partition_broadcast(P))
            return t

        def ln_from_asum(asum, tag, want_nmr=False):
            """mu/rstd (and optionally nmr = -mu*rstd for the Scalar-side
            normalize) from accumulated [s_x(lo), s_x(hi), s_x2(lo),
            s_x2(hi)]."""
            mu = lnp.tile([P, 1], f32, tag=f"mu{tag}")
            nc.vector.tensor_add(out=mu, in0=asum[:, 0:1], in1=asum[:, 1:2])
            nc.vector.tensor_scalar_mul(out=mu, in0=mu, scalar1=1.0 / E)
            s23 = lnp.tile([P, 1], f32, tag=f"s23{tag}")
            nc.vector.tensor_add(out=s23, in0=asum[:, 2:3], in1=asum[:, 3:4])
            mu2 = lnp.tile([P, 1], f32, tag=f"mu2{tag}")
            nc.vector.tensor_mul(out=mu2, in0=mu, in1=mu)
            vr = lnp.tile([P, 1], f32, tag=f"vr{tag}")
            nc.vector.tensor_scalar(out=vr, in0=s23, scalar1=1.0 / E,
                                    scalar2=mu2, op0=MULT, op1=SUB)
            rstd = lnp.tile([P, 1], f32, tag=f"rstd{tag}")
            nc.scalar.activation(out=rstd, in_=vr, func=AF.Sqrt,
                                 bias=eps_sb, scale=1.0)
            nc.vector.reciprocal(out=rstd, in_=rstd)
            if not want_nmr:
                return mu, rstd, None
            nmr = lnp.tile([P, 1], f32, tag=f"nmr{tag}")
            nc.vector.tensor_scalar(out=nmr, in0=mu, scalar1=rstd,
                                    scalar2=-1.0, op0=MULT, op1=MULT)
            return mu, rstd, nmr

        with ExitStack() as phAB:
            qT_pool = phAB.enter_context(tc.tile_pool(name="qT_pool", bufs=1))
            st_pool = phAB.enter_context(tc.tile_pool(name="stp", bufs=2))
            sta_pool = phAB.enter_context(tc.tile_pool(name="stap", bufs=2))
            stb_pool = phAB.enter_context(tc.tile_pool(name="stbp", bufs=2))
            sk_pool = phAB.enter_context(tc.tile_pool(name="skp", bufs=1))
            wp_pool = phAB.enter_context(tc.tile_pool(name="wpp", bufs=1))
            pc_pool = phAB.enter_context(tc.tile_pool(name="pcp", bufs=2))
            ps_s = phAB.enter_context(tc.tile_pool(name="ps_s", bufs=3, space="PSUM"))
            ps_a = phAB.enter_context(tc.tile_pool(name="ps_a", bufs=3, space="PSUM"))
            ps_sum = phAB.enter_context(tc.tile_pool(name="ps_sum", bufs=1, space="PSUM"))

            qT = qT_pool.tile([P, ET, NQ], bf16)
            sk = sk_pool.tile([P, NCH * KT, E], bf16)

            sums = []
            for qs in range(QS):
                sums.append(ps_sum.tile([1, 512], f32, tag=f"sums{qs}",
                                        name=f"sums{qs}"))

            sts, st_a, st_b = {}, {}, {}

            def st_load(cc, eng=None):
                eng = eng or nc.sync
                st = st_pool.tile([P, ET, KCH], bf16, tag="st", name=f"st{cc}")
                eng.dma_start(out=st, in_=srcTt[:, cc])
                sts[cc] = st

            def st_ap(cc, d_t):
                if cc in st_a:
                    return st_a[cc] if d_t == 0 else st_b[cc][:, d_t - 1, :]
                return sts[cc][:, d_t, :]

            def sk_load(cc, eng):
                eng.dma_start(out=sk[:, cc * KT:(cc + 1) * KT, :],
                              in_=srcKt[:, cc * KT:(cc + 1) * KT, :])

            # -- startup-critical DMAs (Sync queue, enqueued at t=0): ONLY
            # what phases A and early-B need, BATCHED into few instructions
            # (each DMA_DIRECT2D costs ~650ns of Sync-engine issue time; 33
            # separate descriptors serialized the whole startup window),
            # split into tiles sized to the consumption order. Everything
            # else enqueues from the Scalar engine's DGE queue at staged
            # points below, so the startup window isn't fair-shared away to
            # prefetches that aren't needed until t+100us.
            # dual-queue startup: Sync and Scalar DGE queues enqueue in
            # parallel (separate DMA engine groups), halving both the
            # ~650ns/instruction issue serialization and per-queue load
            wp0 = wp_pool.tile([P, ET, P], bf16, tag="wp0")
            wp13 = wp_pool.tile([P, 3, ET, P], bf16, tag="wp13")
            wp47 = wp_pool.tile([P, 4, ET, P], bf16, tag="wp47")
            st0a = sta_pool.tile([P, KCH], bf16, tag="sta", name="st0a")
            st0b = stb_pool.tile([P, ET - 1, KCH], bf16, tag="stb", name="st0b")
            st1a = sta_pool.tile([P, KCH], bf16, tag="sta", name="st1a")
            st1b = stb_pool.tile([P, ET - 1, KCH], bf16, tag="stb", name="st1b")
            st_a[0], st_b[0], st_a[1], st_b[1] = st0a, st0b, st1a, st1b
            nc.sync.dma_start(out=wp0, in_=Wpt[:, 0])
            nc.scalar.dma_start(out=st0b, in_=srcTt[:, 0, 1:, :])
            nc.sync.dma_start(out=st0a, in_=srcTt[:, 0, 0, :])
            nc.scalar.dma_start(out=st1a, in_=srcTt[:, 1, 0, :])
            nc.sync.dma_start(out=wp13, in_=Wpt[:, 1:4])
            nc.scalar.dma_start(out=st1b, in_=srcTt[:, 1, 1:, :])
            nc.sync.dma_start(out=wp47, in_=Wpt[:, 4:])
            nc.scalar.dma_start(out=sk[:, 0:KT, :], in_=srcKt[:, 0:KT, :])
            wp_ts = ([wp0] + [wp13[:, i] for i in range(3)]
                     + [wp47[:, i] for i in range(4)])

            # ---------------- Phase A: qT = Wp-contract(srcq) ----------------
            # (e_t=0, qs=1) deferred one iteration so the first matmuls only
            # need st0 -- st1 is still streaming in at that point.
            def qproj(e_t, qs):
                ps = ps_s.tile([P, 512], f32, tag="ps")
                for d_t in range(ET):
                    nc.tensor.matmul(ps, wp_ts[e_t][:, d_t, :], st_ap(qs, d_t),
                                     start=(d_t == 0), stop=(d_t == ET - 1))
                nc.scalar.activation(out=qT[:, e_t, qs * 512:(qs + 1) * 512],
                                     in_=ps, func=AF.Identity,
                                     bias=bqk_sb[:, e_t:e_t + 1], scale=1.0)

            for e_t in range(ET):
                qproj(e_t, 0)
                if e_t == 1:
                    qproj(0, 1)
                if e_t >= 1:
                    qproj(e_t, 1)

            # deferred prefetch, stage 1 (enqueues after A's evictions)
            st_load(2, eng=nc.scalar)
            sk_load(1, nc.scalar)

            # ---------------- Phase B: scores -> exp -> sums/aT --------------
            for cc in range(NCH):
                pc = pc_pool.tile([P, KT, NQ], bf16, tag="pc", name=f"pc{cc}")
                for qs in range(QS):
                    for kt in range(KT):
                        ps = ps_s.tile([P, 512], f32, tag="ps")
                        for d_t in range(ET):
                            nc.tensor.matmul(
                                ps, st_ap(cc, d_t)[:, kt * P:(kt + 1) * P],
                                qT[:, d_t, qs * 512:(qs + 1) * 512],
                                start=(d_t == 0), stop=(d_t == ET - 1))
                        nc.scalar.activation(
                            out=pc[:, kt, qs * 512:(qs + 1) * 512], in_=ps,
                            func=AF.Exp)
                        nc.tensor.matmul(sums[qs], ones_bf,
                                         pc[:, kt, qs * 512:(qs + 1) * 512],
                                         start=(cc == 0 and kt == 0),
                                         stop=(cc == NCH - 1 and kt == KT - 1))
                # deferred prefetch stages 2-4: enqueue from the Scalar
                # queue right after this chunk's exp evictions
                if cc == 0:
                    st_load(3, eng=nc.scalar)
                    sk_load(2, nc.scalar)
                elif cc == 1:
                    sk_load(3, nc.scalar)
                    nc.scalar.dma_start(out=wvo_sb[:, 0:4], in_=Wvot[:, 0:4])
                    nc.scalar.dma_start(out=wvo_sb[:, 4:], in_=Wvot[:, 4:])
                elif cc == 2:
                    nc.scalar.dma_start(out=sq_sb[:, 0:4], in_=srcq[:, 0:4])
                    nc.scalar.dma_start(out=sq_sb[:, 4:], in_=srcq[:, 4:])
                    g1_bc = bcast(g1, E, bcp, nc.scalar)
                    beb2_bc = bcast(beb2, E, bcp, nc.scalar)
                    g2_bc = bcast(g2, E, bcp, nc.scalar)
                    be2_bc = bcast(be2, E, bcp, nc.scalar)
                    w1_pre = []
                    for fc in range(4):
                        w1c = w1_pool.tile([P, ET, 512], f8, tag="w1",
                                           name=f"w1c{fc}")
                        nc.scalar.dma_start(out=w1c, in_=_f8(W1t[:, fc]))
                        w1_pre.append(w1c)
                if cc == NCH - 1:
                    # softmax denominators: spread sums[1, q] across
                    # partitions via K=1 matmuls (1-partition DMAs fail NEFF
                    # load). Hoisted BEFORE the last chunk's aT block so the
                    # rsum chain completes under it and phase C starts hot.
                    for qs in range(QS):
                        nc.vector.tensor_copy(
                            out=sums_sb[:, qs * 512:(qs + 1) * 512],
                            in_=sums[qs])
                    for t in range(ET):
                        pst = ps_a.tile([P, 1], f32, tag="ps", name=f"spread{t}")
                        nc.tensor.matmul(pst, sums_sb[0:1, t * P:(t + 1) * P],
                                         one_sp, start=True, stop=True)
                        nc.vector.tensor_copy(out=rsum[:, t:t + 1], in_=pst)
                    nc.vector.reciprocal(out=rsum, in_=rsum)
                for qs in range(QS):
                    for d_t in range(ET):
                        ps = ps_a.tile([P, 512], f32, tag="ps")
                        for kt in range(KT):
                            nc.tensor.matmul(
                                ps, sk[:, cc * KT + kt, d_t * P:(d_t + 1) * P],
                                pc[:, kt, qs * 512:(qs + 1) * 512],
                                start=(kt == 0), stop=(kt == KT - 1))
                        dst = aT[:, d_t, qs * 512:(qs + 1) * 512]
                        if cc == 0:
                            nc.vector.tensor_copy(out=dst, in_=ps)
                        else:
                            nc.vector.tensor_add(out=dst, in0=dst, in1=ps)

        # late pools: land in the freed attention SBUF; their DMAs start
        # as soon as the aliased phase-A/B tiles' last reads complete
        late = ExitStack()
        try:
            x_pool = late.enter_context(tc.tile_pool(name="x_pool", bufs=1))
            xb_pool = late.enter_context(tc.tile_pool(name="xb_pool", bufs=1))
            xT_pool = late.enter_context(tc.tile_pool(name="xT_pool", bufs=1))
            w2_pool = late.enter_context(tc.tile_pool(name="w2p", bufs=NFC))
            x_sb = x_pool.tile([P, ET, E], f32)    # [q(8x128), e] residual base
            xb16 = xb_pool.tile([P, ET, E], bf16)  # normalized x-hat
            xT = xT_pool.tile([P, ET, NQ], f8)     # [e, q] fp8 for the FF

            w2cs = []
            for fc in range(NFC):
                w2c = w2_pool.tile([P, FCH, E], f8, tag="w2", name=f"w2c{fc}")
                nc.sync.dma_start(out=w2c, in_=_f8(W2t[:, fc]))
                w2cs.append(w2c)

            # ---------- Phase C: x = aT^T@Wvo + residual, LN1, transpose -----
            with ExitStack() as ph3:
                sqs_pool = ph3.enter_context(tc.tile_pool(name="sqs", bufs=2))
                ps_o = ph3.enter_context(tc.tile_pool(name="ps_o", bufs=3, space="PSUM"))
                ps_t = ph3.enter_context(tc.tile_pool(name="ps_t", bufs=2, space="PSUM"))

                def transposes(q_t):
                    """xb16 row q_t -> xT (bf16 PE transposes, fp8 evict)."""
                    for ep in range(ET // 2):
                        pst = ps_t.tile([P, 2, P], bf16, tag="pst")
                        for j in range(2):
                            nc.tensor.transpose(
                                pst[:, j, :],
                                xb16[:, q_t, (2 * ep + j) * P:(2 * ep + j + 1) * P],
                                ident_bf)
                        nc.vector.tensor_copy(
                            out=xT[:, 2 * ep:2 * ep + 2, q_t * P:(q_t + 1) * P],
                            in_=pst)

                for q_t in range(ET):
                    asum = lnp.tile([P, 4], f32, tag="asumC", name=f"asumC{q_t}")
                    for eo in range(2):
                        ps = ps_o.tile([P, 512], f32, tag="ps")
                        for d_t in range(ET):
                            nc.tensor.matmul(ps, aT[:, d_t, q_t * P:(q_t + 1) * P],
                                             wvo_sb[:, d_t, eo * 512:(eo + 1) * 512],
                                             start=(d_t == 0), stop=(d_t == ET - 1))
                        dst = x_sb[:, q_t, eo * 512:(eo + 1) * 512]
                        # x = O*rsum + (src+bo'), one fused Vector op; emits
                        # sum(x) per half for LN1 (bo' = bo + bv@Wo, on host)
                        nc.vector.scalar_tensor_tensor(
                            out=dst, in0=ps, scalar=rsum[:, q_t:q_t + 1],
                            in1=sq_sb[:, q_t, eo * 512:(eo + 1) * 512],
                            op0=MULT, op1=ADD, accum_out=asum[:, eo:eo + 1])
                        sqscr = sqs_pool.tile([P, 512], f32, tag="sqs")
                        nc.scalar.activation(out=sqscr, in_=dst, func=AF.Square,
                                             accum_out=asum[:, 2 + eo:3 + eo])
                    _, rstd, nmr = ln_from_asum(asum, "C", want_nmr=True)
                    # normalize-only LN1 (affine folded into W1/b1'), bf16
                    nc.scalar.activation(out=xb16[:, q_t, :], in_=x_sb[:, q_t, :],
                                         func=AF.Identity, bias=nmr, scale=rstd)
                    # transposes lag one row so Tensor never waits on LN1
                    if q_t > 0:
                        transposes(q_t - 1)
                transposes(ET - 1)

            # ---------------- Phase D: feedforward (fp8 DR) + LN2 -----------
            with ExitStack() as ph4:
                h_pool = ph4.enter_context(tc.tile_pool(name="hp", bufs=1))
                sqs2_pool = ph4.enter_context(tc.tile_pool(name="sqs2", bufs=2))
                ps_h = ph4.enter_context(tc.tile_pool(name="ps_h", bufs=2, space="PSUM"))
                ps_f = ph4.enter_context(tc.tile_pool(name="ps_f", bufs=2, space="PSUM"))

                # rebuild the residual base x-hat*g1 + (be1+b2) in FF1's
                # Vector slack (x_sb rows are free post-normalize)
                for q_t in range(ET):
                    row = x_sb[:, q_t, :]
                    nc.vector.tensor_mul(out=row, in0=xb16[:, q_t, :], in1=g1_bc)
                    nc.vector.tensor_add(out=row, in0=row, in1=beb2_bc)

                # W1 ring refills on the Scalar queue (lookahead 2): the
                # Sync queue head-of-line blocks behind LN2-gated output
                # stores, which would stall FF1(qs1)'s weight loads
                w1s = {g: w1_pre[g] for g in range(4)}

                def w1_refill(g):
                    if g > 15 or g in w1s:
                        return
                    t = w1_pool.tile([P, ET, 512], f8, tag="w1", name=f"w1g{g}")
                    nc.scalar.dma_start(out=t, in_=_f8(W1t[:, g % NFC]))
                    w1s[g] = t

                for qs in range(QS):
                    # FF1 for this query half: h = relu(ps/16 + b1') fp8.
                    # One h buffer (this half only): qs1's evictions wait on
                    # qs0's FF2 reads, which precede them in Tensor order.
                    h_sb = h_pool.tile([P, FT, 512], f8, tag="h", name=f"h{qs}")
                    for fc in range(NFC):
                        g = qs * NFC + fc
                        w1_refill(g + 2)
                        w1c = w1s.pop(g)
                        for fl in range(FCH):
                            f_t = fc * FCH + fl
                            ps = ps_h.tile([P, 512], f32, tag="ps")
                            for dp in range(ET // 2):
                                nc.tensor.matmul(
                                    ps, w1c[:, 2 * dp:2 * dp + 2, fl * P:(fl + 1) * P],
                                    xT[:, 2 * dp:2 * dp + 2, qs * 512:(qs + 1) * 512],
                                    start=(dp == 0), stop=(dp == ET // 2 - 1),
                                    perf_mode=DR)
                            nc.scalar.activation(
                                out=h_sb[:, f_t, :], in_=ps,
                                func=AF.Relu, bias=b1_sb[:, f_t:f_t + 1],
                                scale=1.0 / W1_SCALE)
                    # FF2 + LN2 per 128-row block of this half
                    for q_t in range(qs * (ET // 2), (qs + 1) * (ET // 2)):
                        ql = (q_t - qs * (ET // 2)) * P
                        asum = lnp.tile([P, 4], f32, tag="asumD", name=f"asumD{q_t}")
                        for eo in range(2):
                            ps = ps_f.tile([P, 512], f32, tag="ps")
                            for fc in range(NFC):
                                for fp_ in range(FCH // 2):
                                    ft0 = fc * FCH + 2 * fp_
                                    nc.tensor.matmul(
                                        ps, h_sb[:, ft0:ft0 + 2, ql:ql + P],
                                        w2cs[fc][:, 2 * fp_:2 * fp_ + 2, eo * 512:(eo + 1) * 512],
                                        start=(fc == 0 and fp_ == 0),
                                        stop=(fc == NFC - 1 and fp_ == FCH // 2 - 1),
                                        perf_mode=DR)
                            dst = x_sb[:, q_t, eo * 512:(eo + 1) * 512]
                            # x += ps/W2_SCALE; same op emits sum(x) per half
                            nc.vector.scalar_tensor_tensor(
                                out=dst, in0=ps, scalar=inv_w2s, in1=dst,
                                op0=MULT, op1=ADD, accum_out=asum[:, eo:eo + 1])
                            sqscr = sqs2_pool.tile([P, 512], f32, tag="sqs")
                            nc.scalar.activation(out=sqscr, in_=dst, func=AF.Square,
                                                 accum_out=asum[:, 2 + eo:3 + eo])
                        mu, rstd, _ = ln_from_asum(asum, "D")
                        # fused affine ((x-mu)*g2)*rstd + be2 as Vector STT
                        # pairs; finer slices on the last rows pipeline the
                        # exposed post-matmul chain against the store DMAs
                        nsl = 4 if q_t == ET - 1 else 2
                        w = E // nsl
                        for sl in range(nsl):
                            hs = slice(sl * w, (sl + 1) * w)
                            row = x_sb[:, q_t, hs]
                            nc.vector.scalar_tensor_tensor(
                                out=row, in0=row, scalar=mu, in1=g2_bc[:, hs],
                                op0=SUB, op1=MULT)
                            nc.vector.scalar_tensor_tensor(
                                out=row, in0=row, scalar=rstd, in1=be2_bc[:, hs],
                                op0=MULT, op1=ADD)
                            # alternate store queues: Scalar is idle here and
                            # each DMA instruction costs ~650ns of issue time
                            eng = nc.scalar if sl % 2 else nc.sync
                            eng.dma_start(
                                out=out[q_t * P:(q_t + 1) * P, hs], in_=row)
        finally:
            late.close()

    nc.compile()
    return nc


_NC_CACHE = None


def _pretile(inputs):
    """Host-side weight folds + re-layouts so every DMA is contiguous per
    partition. W1/W2 are scaled and quantized to fp8 e4m3 (uint8 bytes)."""
    import ml_dtypes
    e4m3 = ml_dtypes.float8_e4m3fn
    bf = ml_dtypes.bfloat16
    c = np.ascontiguousarray
    Wq = np.asarray(inputs["Wq"], np.float32)
    Wk = np.asarray(inputs["Wk"], np.float32)
    Wv = np.asarray(inputs["Wv"], np.float32)
    Wo = np.asarray(inputs["Wo"], np.float32)
    # the single-head unscaled-attention folds: scores = srcq@Wp@src^T,
    # attn-out = P_norm@src@Wvo  (see module docstring)
    Wp = Wq @ Wk.T
    Wvo = Wv @ Wo
    bqk = np.asarray(inputs["bq"], np.float32) @ Wk.T
    # fold the LN1 affine into the first FF layer (exact):
    #   (g1*x^ + be1) @ W1 + b1  ==  x^ @ (diag(g1) @ W1) + (b1 + be1 @ W1)
    W1raw = np.asarray(inputs["W1"], np.float32)
    g1v = np.asarray(inputs["g1"], np.float32)
    be1v = np.asarray(inputs["be1"], np.float32)
    W1 = (g1v[:, None] * W1raw) * W1_SCALE
    b1f = np.asarray(inputs["b1"], np.float32) + be1v @ W1raw
    W2 = np.asarray(inputs["W2"], np.float32) * W2_SCALE
    d = {
        "Wpt": c(Wp.reshape(ET, P, ET, P).transpose(1, 2, 0, 3).astype(bf)),
        "Wvot": c(Wvo.reshape(ET, P, E).transpose(1, 0, 2).astype(bf)),
        "W1t": c(W1.reshape(ET, P, NFC, 512).transpose(1, 2, 0, 3)
                 .astype(e4m3)).view(np.uint8),
        "W2t": c(W2.reshape(NFC, FCH, P, E).transpose(2, 0, 1, 3)
                 .astype(e4m3)).view(np.uint8),
        "bqb1": c(np.concatenate(
            [bqk.reshape(ET, P).T, b1f.reshape(FT, P).T], axis=1)),
        "beb2": c(be1v + np.asarray(inputs["b2"], np.float32)),
        "g1": c(g1v),
    }
    for n in ["g2", "be2"]:
        d[n] = c(np.asarray(inputs[n], np.float32))
    return d


def make_in_maps(inputs):
    import ml_dtypes
    src = np.ascontiguousarray(np.asarray(inputs["src"], dtype=np.float32))
    # residual offset: out-projection bias + bv routed through Wo
    # (attention weights sum to 1, so  softmax(S) @ (V0 + bv) @ Wo + bo
    #  == softmax(S) @ V0 @ Wo + (bv @ Wo + bo))
    bo = (np.asarray(inputs["bo"], np.float32)
          + np.asarray(inputs["bv"], np.float32)
          @ np.asarray(inputs["Wo"], np.float32))
    shared = _pretile(inputs)

    in_maps = []
    for core in range(8):
        b, h = core // 2, core % 2
        src_b = src[b]                        # [2048, 1024]
        # permute context so this core's query half is rows/cols 0..1023
        perm = np.concatenate([src_b[h * NQ:(h + 1) * NQ, :],
                               src_b[(1 - h) * NQ:(2 - h) * NQ, :]])
        srcT = perm.T                         # [1024, 2048]
        # [p, c, t, k] = srcT[t*128+p, c*512+k]; bf16 on the wire
        srcTt = np.ascontiguousarray(
            srcT.reshape(ET, P, NCH, KCH).transpose(1, 2, 0, 3)
            .astype(ml_dtypes.bfloat16))
        # k-major copy for the P@src accumulation: [p, ck, e]
        srcKt = np.ascontiguousarray(
            perm.reshape(NCH * KT, P, E).transpose(1, 0, 2)
            .astype(ml_dtypes.bfloat16))
        # bo' (the folded output bias) rides the residual here; bf16 on the
        # wire (0.4% on the residual, small vs the fp8-FF error), pre-tiled
        # partition-major [p, q_t, e] for batched DMA
        srcq = np.ascontiguousarray(
            (src_b[h * NQ:(h + 1) * NQ, :] + bo)
            .reshape(ET, P, E).transpose(1, 0, 2).astype(ml_dtypes.bfloat16))
        in_maps.append({"srcTt": srcTt, "srcKt": srcKt, "srcq": srcq, **shared})
    return in_maps


def gather_out(results):
    out = np.empty((4, S, E), np.float32)
    for core in range(8):
        b, h = core // 2, core % 2
        out[b, h * NQ:(h + 1) * NQ, :] = results[core]["out"]
    return out


def kernel(**inputs):
    global _NC_CACHE
    from concourse.bass_utils import run_bass_kernel_spmd

    in_maps = make_in_maps(inputs)
    if _NC_CACHE is None:
        _NC_CACHE = build_program()
    res = run_bass_kernel_spmd(_NC_CACHE, in_maps, list(range(8)))
    return gather_out(res.results)


if __name__ == "__main__":
    nc = build_program()
    print("build + compile OK")
